# revision 1
# baseline (speedup 1.0000x reference)
"""CrossAttention kernel for 8x TRN2 NeuronCores (Bass/Tile), v2.

Reference computation (per batch b of 16, heads h=8, n=1024, d_model=512, dh=64):
    q = x @ Wq.T, k = x @ Wk.T, v = x @ Wv.T          (per-head slices)
    sim = q k^T * scale + rel_bias[h]
    attn = softmax(sim, axis=-1)
    out = (attn @ v) re-assembled over heads, then @ Wo.T + bo

Sharding: data-parallel over batch, 2 batches per core x 8 cores.

v2 design notes (vs v1 baseline at ~264us modeled):
  - all matmul operands fp16 (full-rate on PE at any tile size, better
    mantissa than bf16, halves weight/x/eb DMA vs f32).
  - softmax runs on transposed sim (j on partitions); rel_bias applied as
    exp(sim)*exp(bias^T) with exp(bias^T) precomputed on host in fp16;
    the multiply runs in-place on DVE in 16-bit 2x mode.
  - attn@V uses V in natural layout as lhsT with an appended ones column:
    the same matmul emits the softmax denominator l as psum row 64.
  - normalization (HW constraint: custom DVE ops and partition_broadcast
    only operate from partition 0, and only on SBUF): DVE-copy the psum
    l-row to SBUF, DMA it to partition 0, reciprocal_approx_fast there,
    gpsimd partition_broadcast, one 1024-wide DVE multiply. Odd heads land
    in AO rows 64..127 via one SBUF->SBUF DMA.
  - output is fp16 (host converts to f32 and adds bias bo) - halves the
    serial output-DMA drain at the end of the program.
  - PSUM split into two rings: "mm" (sim tiles only, so ACT never starves
    behind injected work) and "ot" (attn@V accumulators + all projection /
    output-projection chunks).
  - coarse DMAs: one descriptor-chain per weight matrix / x batch, rel-bias
    loaded in jt-pairs - halves serial HWDGE occupancy.
  - emission is software-pipelined: projections for ip0 first, then the
    attention stream loop with V/QK(ip1..3)/F chunks injected between
    sim slots so the PE never starves while ACT grinds exp.
"""

import numpy as np

HEADS = 8
DH = 64
B = 16
N = 1024
D = 512  # d_model = inner
SCALE = DH ** -0.5
NCORES = 8
BPC = B // NCORES  # batches per core


def build_nc(n=N, bpc=BPC, sim_bufs=2, ot_bufs=2, eb_bufs=7, et_bufs=18,
             lb_bufs=3, fo_bufs=6, tmpo_bufs=2, pool_mul_mod=0, row_copy_act=0, v_copy_act=0, last_norm_split=0, av_flush_slot=0, qk_copy_act_ip=-1, proq_mm=0, f_drain_mm=0, warmup_mms=10, x0_split=0, hoist_sim=1, recip_wide=0, split_inj=0, head_split=1, sched_ip1=(20, 24, 27, 30)):
    import concourse.mybir as mybir
    import concourse.tile as tile
    from concourse import bacc

    f32 = mybir.dt.float32
    f16 = mybir.dt.float16
    Exp = mybir.ActivationFunctionType.Exp
    Copy = mybir.ActivationFunctionType.Copy

    NT = n // 128            # n tiles of 128 (8)
    KP = D // 128            # d_model k-tiles (4)
    HP = HEADS // 2          # head pairs (4)
    T2 = NT // 2             # double-nt chunks (4)

    nc = bacc.Bacc(None, target_bir_lowering=False)

    xT_d = nc.dram_tensor("xT", [bpc, D, n], f16, kind="ExternalInput")
    wq_d = nc.dram_tensor("WqT", [D, D], f16, kind="ExternalInput")   # pre-scaled
    wk_d = nc.dram_tensor("WkT", [D, D], f16, kind="ExternalInput")
    wv_d = nc.dram_tensor("WvT", [D, D], f16, kind="ExternalInput")
    wo_d = nc.dram_tensor("WoT", [D, D], f16, kind="ExternalInput")
    eb_d = nc.dram_tensor("expBT", [HEADS, n, n], f16, kind="ExternalInput")
    out_d = nc.dram_tensor("out", [bpc, n, D], f16, kind="ExternalOutput")

    with tile.TileContext(nc) as tc:
        pers = tc.alloc_tile_pool(name="pers", bufs=1)
        # ---- persistent tiles
        QT = [[pers.tile([128, n], f16, tag=f"qt{bi}_{ip}", name=f"qt{bi}_{ip}")
               for ip in range(KP)] for bi in range(bpc)]
        KT = [[pers.tile([128, n], f16, tag=f"kt{bi}_{ip}", name=f"kt{bi}_{ip}")
               for ip in range(KP)] for bi in range(bpc)]
        VA = [[pers.tile([128, HEADS * (DH + 1)], f16, tag=f"va{bi}_{nt}",
                         name=f"va{bi}_{nt}") for nt in range(NT)]
              for bi in range(bpc)]
        AO = [[pers.tile([128, n], f16, tag=f"ao{bi}_{kp}", name=f"ao{bi}_{kp}")
               for kp in range(KP)] for bi in range(bpc)]
        # each weight matrix lives in one [128, KP, 512] tile (one DMA each)
        w4 = {}
        for wname, wd in (("q", wq_d), ("k", wk_d), ("v", wv_d), ("o", wo_d)):
            t = pers.tile([128, KP, D], f16, tag=f"w{wname}", name=f"w{wname}")
            w4[wname] = t
        xtp = tc.alloc_tile_pool(name="xt", bufs=1)
        xt = [xtp.tile([128, KP, n], f16, tag=f"x{bi}", name=f"x{bi}")
              for bi in range(bpc)]

        ebp = tc.alloc_tile_pool(name="eb", bufs=eb_bufs)
        etp = tc.alloc_tile_pool(name="et", bufs=et_bufs)
        lrp = tc.alloc_tile_pool(name="lr", bufs=lb_bufs)
        fop = tc.alloc_tile_pool(name="fop", bufs=1)
        ps = tc.alloc_tile_pool(name="ps", bufs=1, space="PSUM")

        # ---- prologue DMAs (one chained DMA per tensor)
        def dma_w(wname, wd):
            dst = w4[wname][:]
            src = wd.rearrange("(kp p) c -> p kp c", p=128)
            nc.sync.dma_start(out=dst, in_=src)

        def dma_x(bi, split=False):
            if split:
                for kp in range(KP):
                    nc.sync.dma_start(
                        out=xt[bi][:, kp, :],
                        in_=xT_d[bi, kp * 128:(kp + 1) * 128, :])
            else:
                nc.sync.dma_start(
                    out=xt[bi][:], in_=xT_d[bi].rearrange("(kp p) j -> p kp j", p=128))

        pool_jts = set(range(3, 3 + pool_mul_mod))
        av_jt_order = [j for j in range(NT) if j not in pool_jts] + sorted(pool_jts)
        eb_tiles = {}

        def emit_eb_dma(h, jp):
            """Load jt pair (2*jp, 2*jp+1) of head h as one [128, 2, n] tile."""
            t = ebp.tile([128, 2, n], f16, tag="eb", name="eb")
            nc.sync.dma_start(
                out=t[:],
                in_=eb_d[h, 2 * jp * 128:(2 * jp + 2) * 128, :].rearrange(
                    "(two p) i -> p two i", two=2))
            eb_tiles[(h, 2 * jp)] = t[:, 0, :]
            eb_tiles[(h, 2 * jp + 1)] = t[:, 1, :]

        dma_w("q", wq_d)
        dma_x(0, split=bool(x0_split))
        dma_w("k", wk_d)
        dma_x(1)
        dma_w("v", wv_d)
        dma_w("o", wo_d)
        for jp in range(NT // 2):
            emit_eb_dma(0, jp)

        # ---- PE warm-up: keep the tensor engine continuously busy through
        # the prologue DMAs so its p-state clock is fully ramped (213ns/row
        # instead of 427+) when the first real projection matmuls arrive.
        if warmup_mms:
            scr = pers.tile([128, 512], f16, tag="scr", name="scr")
            nc.gpsimd.memset(scr[:], 0.0)
            wps = ps.tile([128, 512], f32, tag="mm", bufs=sim_bufs, name="wps")
            for _ in range(warmup_mms):
                nc.tensor.matmul(wps[:], scr[:, 0:128], scr[:],
                                 start=True, stop=True)

        # ---- emitters -----------------------------------------------------
        inj_ring = [0]

        def inj_tag():
            return "ot"

        def emit_qk_chunk(wname, DST, bi, ip, ring=None, halves=(0, 1), first_split=False):
            W_s = w4[wname]
            wide = len(halves) == 2 and not split_inj
            pt = ps.tile([128, 1024 if wide else 512], f32,
                         tag=ring or inj_tag(), bufs=ot_bufs, name="pt")
            for i, nh in enumerate(halves):
                base = i * 512 if wide else 0
                for kp in range(KP):
                    nc.tensor.matmul(
                        pt[:, base:base + 512],
                        W_s[:, kp, ip * 128:(ip + 1) * 128],
                        xt[bi][:, kp, nh * 512:(nh + 1) * 512],
                        start=(kp == 0), stop=(kp == KP - 1),
                    )
                if not wide:
                    nc.vector.tensor_copy(
                        out=DST[bi][ip][:, nh * 512:(nh + 1) * 512], in_=pt[:])
            if wide:
                if first_split:
                    # sim jt0 needs only K cols 0:128 / Q cols 0:512 - copy
                    # those first so the first sim fires sooner.
                    c0 = 128 if wname == "k" else 512
                    nc.vector.tensor_copy(out=DST[bi][ip][:, 0:c0], in_=pt[:, 0:c0])
                    nc.vector.tensor_copy(out=DST[bi][ip][:, c0:], in_=pt[:, c0:])
                else:
                    nc.vector.tensor_copy(out=DST[bi][ip][:], in_=pt[:])

        def emit_v_chunk(bi, t2, blocks=(0, 1)):
            wide = len(blocks) == 2 and not split_inj
            pt = ps.tile([128, 1024 if wide else 512], f32, tag=inj_tag(),
                         bufs=ot_bufs, name="pt")
            for i, b in enumerate(blocks):
                nt = 2 * t2 + b
                base = i * 512 if wide else 0
                for kp in range(KP):
                    nc.tensor.matmul(
                        pt[:, base:base + 512],
                        xt[bi][:, kp, nt * 128:(nt + 1) * 128],
                        w4["v"][:, kp, :],
                        start=(kp == 0), stop=(kp == KP - 1),
                    )
                va = VA[bi][nt]
                nc.gpsimd.memset(va[:], 1.0)
                dst3 = va[:].rearrange("p (h c) -> p h c", c=DH + 1)[:, :, 0:DH]
                src3 = pt[:, base:base + 512].rearrange("p (h c) -> p h c", c=DH)
                nc.vector.tensor_copy(out=dst3, in_=src3)

        def emit_f_chunk(bi, t2, ring=None):
            pt = ps.tile([128, 1024], f32, tag=ring or inj_tag(), bufs=ot_bufs,
                         name="pt")
            for b in range(2):
                nt = 2 * t2 + b
                for kp in range(KP):
                    nc.tensor.matmul(
                        pt[:, b * 512:(b + 1) * 512],
                        AO[bi][kp][:, nt * 128:(nt + 1) * 128],
                        w4["o"][:, kp, :],
                        start=(kp == 0), stop=(kp == KP - 1),
                    )
            for b in range(2):
                nt = 2 * t2 + b
                fo = fop.tile([128, 512], f16, tag="fo", bufs=fo_bufs, name="fo")
                nc.scalar.activation(fo[:], pt[:, b * 512:(b + 1) * 512], Copy)
                nc.sync.dma_start(out=out_d[bi, nt * 128:(nt + 1) * 128, :], in_=fo[:])

        pend = {"av": None}

        def start_av(h, bi, et_row):
            ot = ps.tile([DH + 1, 1024], f32, tag="ot", bufs=ot_bufs, name="ot")
            mms = [(ihh, jt) for ihh in range(2) for jt in av_jt_order]
            pend["av"] = {"h": h, "bi": bi, "et": et_row, "ot": ot, "mms": mms}

        def step_av(nmm):
            st = pend["av"]
            if st is None:
                return
            h, bi, et_row, ot = st["h"], st["bi"], st["et"], st["ot"]
            while nmm > 0 and st["mms"]:
                ihh, jt = st["mms"].pop(0)
                nc.tensor.matmul(
                    ot[:, ihh * 512:(ihh + 1) * 512],
                    VA[bi][jt][:, h * (DH + 1):(h + 1) * (DH + 1)],
                    et_row[jt][:, ihh * 512:(ihh + 1) * 512],
                    start=(jt == av_jt_order[0]), stop=(jt == av_jt_order[-1]),
                )
                nmm -= 1
            if not st["mms"]:
                emit_norm(h, bi, ot)
                pend["av"] = None

        norm_idx = [0]

        def emit_norm(h, bi, ot, half=None):
            # HW-validated chain: copy the psum l-row (partition 64) to SBUF,
            # DMA it to partition 0, reciprocal there, broadcast from
            # partition 0. Custom DVE ops and partition_broadcast only work
            # from partition 0 on real hardware; ACT can read psum anywhere.
            sl = slice(None) if half is None else slice(half * 512, (half + 1) * 512)
            w = n if half is None else 512
            lb = lrp.tile([DH + 1, n], f32, tag="lb", name="lb") if half != 1 \
                else emit_norm.lb_cur
            emit_norm.lb_cur = lb
            if norm_idx[0] % 2 < row_copy_act:
                nc.scalar.activation(lb[DH:DH + 1, sl], ot[DH:DH + 1, sl], Copy)
            else:
                nc.vector.tensor_copy(out=lb[DH:DH + 1, sl], in_=ot[DH:DH + 1, sl])
            norm_idx[0] += 1
            lr0 = lrp.tile([1, n], f32, tag="lr0", bufs=1, name="lr0") if half != 1 \
                else emit_norm.lr0_cur
            emit_norm.lr0_cur = lr0
            nc.sync.dma_start(out=lr0[0:1, sl], in_=lb[DH:DH + 1, sl])
            lrr = lrp.tile([1, n], f32, tag="lrr", bufs=1, name="lrr") if half != 1 \
                else emit_norm.lrr_cur
            emit_norm.lrr_cur = lrr
            nc.vector.reciprocal_approx_fast(out=lrr[0:1, sl], in_=lr0[0:1, sl])
            nc.gpsimd.partition_broadcast(lb[0:DH, sl], lrr[0:1, sl], channels=DH)
            if h % 2 == 0:
                nc.vector.tensor_mul(
                    out=AO[bi][h // 2][0:DH, sl], in0=ot[0:DH, sl], in1=lb[0:DH, sl])
            else:
                tmpo = lrp.tile([DH, n], f16, tag="tmpo", bufs=tmpo_bufs, name="tmpo") \
                    if half != 1 else emit_norm.tmpo_cur
                emit_norm.tmpo_cur = tmpo
                nc.vector.tensor_mul(out=tmpo[0:DH, sl], in0=ot[0:DH, sl], in1=lb[0:DH, sl])
                nc.sync.dma_start(out=AO[bi][h // 2][DH:128, sl], in_=tmpo[0:DH, sl])

        # ---- prologue compute: QK projections for ip=0 (heads 0,1).
        # The sim ring is empty this early, so borrow it: four chunks then
        # rotate through four psum slots instead of two.
        for bi in range(bpc):
            emit_qk_chunk("q", QT, bi, 0, ring="mm" if proq_mm else None,
                          first_split=(bi == 0 and bool(head_split)))
            emit_qk_chunk("k", KT, bi, 0, ring="mm" if proq_mm else None,
                          first_split=(bi == 0 and bool(head_split)))

        # ---- deadline-scheduled inject: chunk -> emission slot
        sched_ip1 = list(sched_ip1)
        sched = {}
        slots_v0 = [0, 2, 4, 6]
        slots_v1 = [8, 10, 12, 14]
        slots_ip = {1: sched_ip1, 2: [36, 44, 52, 60], 3: [68, 76, 84, 92]}
        if split_inj:
            for t2 in range(T2):
                sched[slots_v0[t2]] = [("vh", 0, t2, 0), ("vh", 0, t2, 1)]
                sched[slots_v1[t2]] = [("vh", 1, t2, 0), ("vh", 1, t2, 1)]
            for ip in range(1, KP):
                for i, (kind, bi) in enumerate((("q", 0), ("k", 0), ("q", 1), ("k", 1))):
                    sched[slots_ip[ip][i]] = [(kind + "h", bi, ip, 0),
                                              (kind + "h", bi, ip, 1)]
        else:
            for t2 in range(T2):
                sched[slots_v0[t2]] = [("v", 0, t2)]
                sched[slots_v1[t2]] = [("v", 1, t2)]
            for ip in range(1, KP):
                for i, (kind, bi) in enumerate((("q", 0), ("k", 0), ("q", 1), ("k", 1))):
                    sched[slots_ip[ip][i]] = [(kind, bi, ip)]
        for t2 in range(T2):
            sched[120 + 2 * t2] = [("f", 0, t2)]

        def emit_chunk(c):
            kind = c[0]
            if kind == "v":
                emit_v_chunk(c[1], c[2])
            elif kind == "vh":
                emit_v_chunk(c[1], c[2], blocks=(c[3],))
            elif kind == "q":
                emit_qk_chunk("q", QT, c[1], c[2])
            elif kind == "k":
                emit_qk_chunk("k", KT, c[1], c[2])
            elif kind == "qh":
                emit_qk_chunk("q", QT, c[1], c[2], halves=(c[3],))
            elif kind == "kh":
                emit_qk_chunk("k", KT, c[1], c[2], halves=(c[3],))
            elif kind == "f":
                emit_f_chunk(c[1], c[2])

        # ---- main attention loop; last head pair runs h-odd first so the
        # final stream is an even head (its norm-mul writes AO directly,
        # shortening the tail by one SBUF->SBUF DMA hop).
        stream_order = {}
        for hp in range(HP):
            h0, h1 = 2 * hp, 2 * hp + 1
            if hp == HP - 1:
                stream_order[hp] = [(h1, 0), (h1, 1), (h0, 0), (h0, 1)]
            else:
                stream_order[hp] = [(h0, 0), (h0, 1), (h1, 0), (h1, 1)]

        slot = 0
        for hp in range(HP):
            streams = stream_order[hp]
            first_h = streams[0][0]
            other_h = streams[2][0]
            for sidx, (h, bi) in enumerate(streams):
                is_last = (hp == HP - 1 and sidx == 3)
                ot_last = None
                if is_last:
                    ot_last = ps.tile([DH + 1, 1024], f32, tag="ot",
                                      bufs=ot_bufs, name="ot_last")
                et_row = []
                sp_hoist = [None]
                for jt in range(NT):
                    # eb prefetch (jt pairs): sidx1 fetches this pair's other
                    # head, sidx2 fetches the next head-pair's first head.
                    if jt % 2 == 0:
                        if sidx == 1:
                            emit_eb_dma(other_h, jt // 2)
                        elif sidx == 2 and hp + 1 < HP:
                            emit_eb_dma(stream_order[hp + 1][0][0], jt // 2)
                    po = (h % 2) * 64

                    def emit_sim(j):
                        spj = ps.tile([128, 1024], f32, tag="mm", bufs=sim_bufs,
                                      name="spj")
                        for ihh in range(2):
                            nc.tensor.matmul(
                                spj[:, ihh * 512:(ihh + 1) * 512],
                                KT[bi][hp][po:po + 64, j * 128:(j + 1) * 128],
                                QT[bi][hp][po:po + 64, ihh * 512:(ihh + 1) * 512],
                                start=True, stop=True,
                            )
                        return spj

                    if jt == 1 and sp_hoist[0] is not None:
                        sp = sp_hoist[0]
                    else:
                        sp = emit_sim(jt)
                    if hoist_sim and jt == 0 and pend["av"] is not None:
                        sp_hoist[0] = emit_sim(1)
                    eq = etp.tile([128, n], f16, tag="eq", bufs=8, name="eq")
                    nc.scalar.activation(eq[:], sp[:], Exp)
                    et = etp.tile([128, n], f16, tag="et", name="et")
                    use_pool = (not is_last) and jt in pool_jts
                    mul_eng = nc.gpsimd if use_pool else nc.vector
                    mul_eng.tensor_mul(out=et[:], in0=eq[:], in1=eb_tiles[(h, jt)])
                    et_row.append(et)
                    if is_last:
                        for ihh in range(2):
                            nc.tensor.matmul(
                                ot_last[:, ihh * 512:(ihh + 1) * 512],
                                VA[bi][jt][:, h * (DH + 1):(h + 1) * (DH + 1)],
                                et[:, ihh * 512:(ihh + 1) * 512],
                                start=(jt == 0), stop=(jt == NT - 1),
                            )
                    if jt == av_flush_slot:
                        step_av(99)
                    if slot in sched:
                        for c in sched.pop(slot):
                            emit_chunk(c)
                    slot += 1
                step_av(99)  # flush any remaining AV of the previous stream
                if is_last:
                    if last_norm_split:
                        emit_norm(h, bi, ot_last, half=0)
                        emit_norm(h, bi, ot_last, half=1)
                    else:
                        emit_norm(h, bi, ot_last)
                else:
                    start_av(h, bi, et_row)
        step_av(99)
        # drain: anything left, then F for batch 1
        for s in sorted(sched):
            for c in sched[s]:
                emit_chunk(c)
        for t2 in range(T2):
            emit_f_chunk(1, t2, ring=("mm" if (f_drain_mm and t2 % 2) else None))

        for p in (ps, fop, lrp, etp, ebp, xtp, pers):
            p.release()

    nc.compile()
    return nc


def prep_inputs(x, Wq, Wk, Wv, rel_bias, Wo, bo, n=N, bpc=BPC, ncores=NCORES):
    """Host-side sharding/layout prep. Returns in_maps (one dict per core)."""
    f16 = np.float16
    x = np.asarray(x, dtype=np.float32)
    xT = np.ascontiguousarray(x.transpose(0, 2, 1)).astype(f16)   # [B, D, n]
    WqT = np.ascontiguousarray(Wq.T * np.float32(SCALE)).astype(f16)
    WkT = np.ascontiguousarray(Wk.T).astype(f16)
    WvT = np.ascontiguousarray(Wv.T).astype(f16)
    WoT = np.ascontiguousarray(Wo.T).astype(f16)
    expBT = np.ascontiguousarray(
        np.exp(np.asarray(rel_bias, dtype=np.float32).transpose(0, 2, 1))
    ).astype(f16)                                                  # [H, n(j), n(i)]
    in_maps = []
    for c in range(ncores):
        in_maps.append({
            "xT": np.ascontiguousarray(xT[c * bpc:(c + 1) * bpc]),
            "WqT": WqT, "WkT": WkT, "WvT": WvT, "WoT": WoT,
            "expBT": expBT,
        })
    return in_maps


_CACHE = {}


def kernel(x, Wq, Wk, Wv, rel_bias, Wo, bo):
    from concourse.bass_utils import run_bass_kernel_spmd

    if "nc" not in _CACHE:
        _CACHE["nc"] = build_nc()
    nc = _CACHE["nc"]
    in_maps = prep_inputs(x, Wq, Wk, Wv, rel_bias, Wo, bo)
    res = run_bass_kernel_spmd(nc, in_maps, core_ids=list(range(NCORES)))
    out = np.concatenate([res.results[c]["out"] for c in range(NCORES)], axis=0)
    out = out + np.asarray(bo, dtype=np.float32)[None, None, :]
    return np.ascontiguousarray(out, dtype=np.float32)



# revision 17
# speedup vs baseline: 1.0144x; 1.0144x over previous
"""CrossAttention kernel for 8x TRN2 NeuronCores (Bass/Tile), v2.

Reference computation (per batch b of 16, heads h=8, n=1024, d_model=512, dh=64):
    q = x @ Wq.T, k = x @ Wk.T, v = x @ Wv.T          (per-head slices)
    sim = q k^T * scale + rel_bias[h]
    attn = softmax(sim, axis=-1)
    out = (attn @ v) re-assembled over heads, then @ Wo.T + bo

Sharding: data-parallel over batch, 2 batches per core x 8 cores.

v2 design notes (vs v1 baseline at ~264us modeled):
  - all matmul operands fp16 (full-rate on PE at any tile size, better
    mantissa than bf16, halves weight/x/eb DMA vs f32).
  - softmax runs on transposed sim (j on partitions); rel_bias applied as
    exp(sim)*exp(bias^T) with exp(bias^T) precomputed on host in fp16;
    the multiply runs in-place on DVE in 16-bit 2x mode.
  - attn@V uses V in natural layout as lhsT with an appended ones column:
    the same matmul emits the softmax denominator l as psum row 64.
  - normalization (HW constraint: custom DVE ops and partition_broadcast
    only operate from partition 0, and only on SBUF): DVE-copy the psum
    l-row to SBUF, DMA it to partition 0, reciprocal_approx_fast there,
    gpsimd partition_broadcast, one 1024-wide DVE multiply. Odd heads land
    in AO rows 64..127 via one SBUF->SBUF DMA.
  - output is fp16 (host converts to f32 and adds bias bo) - halves the
    serial output-DMA drain at the end of the program.
  - PSUM split into two rings: "mm" (sim tiles only, so ACT never starves
    behind injected work) and "ot" (attn@V accumulators + all projection /
    output-projection chunks).
  - coarse DMAs: one descriptor-chain per weight matrix / x batch, rel-bias
    loaded in jt-pairs - halves serial HWDGE occupancy.
  - emission is software-pipelined: projections for ip0 first, then the
    attention stream loop with V/QK(ip1..3)/F chunks injected between
    sim slots so the PE never starves while ACT grinds exp.
"""

import numpy as np

HEADS = 8
DH = 64
B = 16
N = 1024
D = 512  # d_model = inner
SCALE = DH ** -0.5
NCORES = 8
BPC = B // NCORES  # batches per core


def build_nc(n=N, bpc=BPC, sim_bufs=2, ot_bufs=2, eb_bufs=7, et_bufs=18,
             lb_bufs=3, fo_bufs=6, tmpo_bufs=2, pool_mul_mod=0, row_copy_act=0, v_copy_act=0, last_norm_split=0, av_flush_slot=0, qk_copy_act_ip=-1, proq_mm=0, f_drain_mm=0, warmup_mms=10, x0_split=0, hoist_sim=1, recip_wide=0, split_inj=0, head_split=1, sched_ip1=(20, 24, 27, 30),
             f_copy_dve=0, recip_scatter=0, inline_av_lag=0, drain_fine=0,
             norm_fast=0, scr_dve=0):
    import concourse.mybir as mybir
    import concourse.tile as tile
    from concourse import bacc

    f32 = mybir.dt.float32
    f16 = mybir.dt.float16
    Exp = mybir.ActivationFunctionType.Exp
    Copy = mybir.ActivationFunctionType.Copy

    NT = n // 128            # n tiles of 128 (8)
    KP = D // 128            # d_model k-tiles (4)
    HP = HEADS // 2          # head pairs (4)
    T2 = NT // 2             # double-nt chunks (4)

    nc = bacc.Bacc(None, target_bir_lowering=False)

    xT_d = nc.dram_tensor("xT", [bpc, D, n], f16, kind="ExternalInput")
    wq_d = nc.dram_tensor("WqT", [D, D], f16, kind="ExternalInput")   # pre-scaled
    wk_d = nc.dram_tensor("WkT", [D, D], f16, kind="ExternalInput")
    wv_d = nc.dram_tensor("WvT", [D, D], f16, kind="ExternalInput")
    wo_d = nc.dram_tensor("WoT", [D, D], f16, kind="ExternalInput")
    eb_d = nc.dram_tensor("expBT", [HEADS, n, n], f16, kind="ExternalInput")
    out_d = nc.dram_tensor("out", [bpc, n, D], f16, kind="ExternalOutput")

    with tile.TileContext(nc) as tc:
        pers = tc.alloc_tile_pool(name="pers", bufs=1)
        # ---- persistent tiles
        QT = [[pers.tile([128, n], f16, tag=f"qt{bi}_{ip}", name=f"qt{bi}_{ip}")
               for ip in range(KP)] for bi in range(bpc)]
        KT = [[pers.tile([128, n], f16, tag=f"kt{bi}_{ip}", name=f"kt{bi}_{ip}")
               for ip in range(KP)] for bi in range(bpc)]
        VA = [[pers.tile([128, HEADS * (DH + 1)], f16, tag=f"va{bi}_{nt}",
                         name=f"va{bi}_{nt}") for nt in range(NT)]
              for bi in range(bpc)]
        AO = [[pers.tile([128, n], f16, tag=f"ao{bi}_{kp}", name=f"ao{bi}_{kp}")
               for kp in range(KP)] for bi in range(bpc)]
        # each weight matrix lives in one [128, KP, 512] tile (one DMA each)
        w4 = {}
        for wname, wd in (("q", wq_d), ("k", wk_d), ("v", wv_d), ("o", wo_d)):
            t = pers.tile([128, KP, D], f16, tag=f"w{wname}", name=f"w{wname}")
            w4[wname] = t
        xtp = tc.alloc_tile_pool(name="xt", bufs=1)
        xt = [xtp.tile([128, KP, n], f16, tag=f"x{bi}", name=f"x{bi}")
              for bi in range(bpc)]

        ebp = tc.alloc_tile_pool(name="eb", bufs=eb_bufs)
        etp = tc.alloc_tile_pool(name="et", bufs=et_bufs)
        lrp = tc.alloc_tile_pool(name="lr", bufs=lb_bufs)
        fop = tc.alloc_tile_pool(name="fop", bufs=1)
        ps = tc.alloc_tile_pool(name="ps", bufs=1, space="PSUM")

        # ---- prologue DMAs (one chained DMA per tensor)
        def dma_w(wname, wd):
            dst = w4[wname][:]
            src = wd.rearrange("(kp p) c -> p kp c", p=128)
            nc.sync.dma_start(out=dst, in_=src)

        def dma_x(bi, split=False):
            if split:
                for kp in range(KP):
                    nc.sync.dma_start(
                        out=xt[bi][:, kp, :],
                        in_=xT_d[bi, kp * 128:(kp + 1) * 128, :])
            else:
                nc.sync.dma_start(
                    out=xt[bi][:], in_=xT_d[bi].rearrange("(kp p) j -> p kp j", p=128))

        pool_jts = set(range(3, 3 + pool_mul_mod))
        av_jt_order = [j for j in range(NT) if j not in pool_jts] + sorted(pool_jts)
        eb_tiles = {}

        def emit_eb_dma(h, jp):
            """Load jt pair (2*jp, 2*jp+1) of head h as one [128, 2, n] tile."""
            t = ebp.tile([128, 2, n], f16, tag="eb", name="eb")
            nc.sync.dma_start(
                out=t[:],
                in_=eb_d[h, 2 * jp * 128:(2 * jp + 2) * 128, :].rearrange(
                    "(two p) i -> p two i", two=2))
            eb_tiles[(h, 2 * jp)] = t[:, 0, :]
            eb_tiles[(h, 2 * jp + 1)] = t[:, 1, :]

        dma_w("q", wq_d)
        dma_x(0, split=bool(x0_split))
        dma_w("k", wk_d)
        dma_x(1)
        dma_w("v", wv_d)
        dma_w("o", wo_d)
        for jp in range(NT // 2):
            emit_eb_dma(0, jp)

        # ---- PE warm-up: keep the tensor engine continuously busy through
        # the prologue DMAs so its p-state clock is fully ramped (213ns/row
        # instead of 427+) when the first real projection matmuls arrive.
        if warmup_mms:
            scr = pers.tile([128, 512], f16, tag="scr", name="scr")
            if scr_dve:
                nc.vector.memset(scr[:], 0.0)
            else:
                nc.gpsimd.memset(scr[:], 0.0)
            wps = ps.tile([128, 512], f32, tag="mm", bufs=sim_bufs, name="wps")
            for _ in range(warmup_mms):
                nc.tensor.matmul(wps[:], scr[:, 0:128], scr[:],
                                 start=True, stop=True)

        # ---- emitters -----------------------------------------------------
        inj_ring = [0]

        def inj_tag():
            return "ot"

        def emit_qk_chunk(wname, DST, bi, ip, ring=None, halves=(0, 1), first_split=False):
            W_s = w4[wname]
            wide = len(halves) == 2 and not split_inj
            pt = ps.tile([128, 1024 if wide else 512], f32,
                         tag=ring or inj_tag(), bufs=ot_bufs, name="pt")
            for i, nh in enumerate(halves):
                base = i * 512 if wide else 0
                for kp in range(KP):
                    nc.tensor.matmul(
                        pt[:, base:base + 512],
                        W_s[:, kp, ip * 128:(ip + 1) * 128],
                        xt[bi][:, kp, nh * 512:(nh + 1) * 512],
                        start=(kp == 0), stop=(kp == KP - 1),
                    )
                if not wide:
                    nc.vector.tensor_copy(
                        out=DST[bi][ip][:, nh * 512:(nh + 1) * 512], in_=pt[:])
            if wide:
                if first_split:
                    # sim jt0 needs only K cols 0:128 / Q cols 0:512 - copy
                    # those first so the first sim fires sooner.
                    c0 = 128 if wname == "k" else 512
                    nc.vector.tensor_copy(out=DST[bi][ip][:, 0:c0], in_=pt[:, 0:c0])
                    nc.vector.tensor_copy(out=DST[bi][ip][:, c0:], in_=pt[:, c0:])
                else:
                    nc.vector.tensor_copy(out=DST[bi][ip][:], in_=pt[:])

        def emit_v_chunk(bi, t2, blocks=(0, 1)):
            wide = len(blocks) == 2 and not split_inj
            pt = ps.tile([128, 1024 if wide else 512], f32, tag=inj_tag(),
                         bufs=ot_bufs, name="pt")
            for i, b in enumerate(blocks):
                nt = 2 * t2 + b
                base = i * 512 if wide else 0
                for kp in range(KP):
                    nc.tensor.matmul(
                        pt[:, base:base + 512],
                        xt[bi][:, kp, nt * 128:(nt + 1) * 128],
                        w4["v"][:, kp, :],
                        start=(kp == 0), stop=(kp == KP - 1),
                    )
                va = VA[bi][nt]
                nc.gpsimd.memset(va[:], 1.0)
                dst3 = va[:].rearrange("p (h c) -> p h c", c=DH + 1)[:, :, 0:DH]
                src3 = pt[:, base:base + 512].rearrange("p (h c) -> p h c", c=DH)
                nc.vector.tensor_copy(out=dst3, in_=src3)

        def emit_f_drain(pt, bi, nt, b):
            fo = fop.tile([128, 512], f16, tag="fo", bufs=fo_bufs, name="fo")
            if f_copy_dve:
                nc.vector.tensor_copy(out=fo[:], in_=pt[:, b * 512:(b + 1) * 512])
            else:
                nc.scalar.activation(fo[:], pt[:, b * 512:(b + 1) * 512], Copy)
            nc.sync.dma_start(out=out_d[bi, nt * 128:(nt + 1) * 128, :], in_=fo[:])

        def emit_f_chunk(bi, t2, ring=None, fine=False):
            pt = ps.tile([128, 1024], f32, tag=ring or inj_tag(), bufs=ot_bufs,
                         name="pt")
            for b in range(2):
                nt = 2 * t2 + b
                for kp in range(KP):
                    nc.tensor.matmul(
                        pt[:, b * 512:(b + 1) * 512],
                        AO[bi][kp][:, nt * 128:(nt + 1) * 128],
                        w4["o"][:, kp, :],
                        start=(kp == 0), stop=(kp == KP - 1),
                    )
                if fine:
                    emit_f_drain(pt, bi, nt, b)
            if not fine:
                for b in range(2):
                    emit_f_drain(pt, bi, 2 * t2 + b, b)

        pend_avs = []

        def pend_any():
            return bool(pend_avs)

        def start_av(h, bi, et_row, inline=False):
            ot = ps.tile([DH + 1, 1024], f32, tag="ot", bufs=ot_bufs, name="ot")
            if inline:
                mms = [(ihh, jt) for jt in av_jt_order for ihh in range(2)]
            else:
                mms = [(ihh, jt) for ihh in range(2) for jt in av_jt_order]
            pend_avs.append({"h": h, "bi": bi, "et": et_row, "ot": ot,
                             "mms": mms, "lag": inline_av_lag if inline else 0})

        def step_av(nmm):
            while nmm > 0 and pend_avs:
                st = pend_avs[0]
                h, bi, et_row, ot = st["h"], st["bi"], st["et"], st["ot"]
                if not st["mms"]:
                    pend_avs.pop(0)
                    emit_norm(h, bi, ot)
                    continue
                ihh, jt = st["mms"][0]
                lag = st["lag"] if st is pend_avs[-1] else 0
                if jt >= len(et_row) - lag:
                    return
                st["mms"].pop(0)
                nc.tensor.matmul(
                    ot[:, ihh * 512:(ihh + 1) * 512],
                    VA[bi][jt][:, h * (DH + 1):(h + 1) * (DH + 1)],
                    et_row[jt][:, ihh * 512:(ihh + 1) * 512],
                    start=(jt == av_jt_order[0]), stop=(jt == av_jt_order[-1]),
                )
                nmm -= 1
                if not st["mms"]:
                    pend_avs.pop(0)
                    emit_norm(h, bi, ot)

        norm_idx = [0]

        def emit_norm(h, bi, ot, half=None):
            if norm_fast:
                # Short chain: cross-partition-base DVE copy moves the psum
                # l-row (partition 64) straight to a partition-0 SBUF row (no
                # DMA hop), reciprocal there, broadcast, and the apply writes
                # odd-head AO rows at partition base 64 directly (no tmpo DMA).
                sl = slice(None) if half is None else slice(half * 512, (half + 1) * 512)
                lbf = lrp.tile([DH, n], f32, tag="lb", name="lbf") if half != 1 \
                    else emit_norm.lb_cur
                emit_norm.lb_cur = lbf
                lrow = lrp.tile([1, n], f32, tag="lrow", bufs=2, name="lrow") if half != 1 \
                    else emit_norm.lrow_cur
                emit_norm.lrow_cur = lrow
                nc.vector.tensor_copy(out=lrow[0:1, sl], in_=ot[DH:DH + 1, sl])
                nc.vector.reciprocal_approx_fast(out=lrow[0:1, sl], in_=lrow[0:1, sl])
                nc.gpsimd.partition_broadcast(lbf[0:DH, sl], lrow[0:1, sl], channels=DH)
                po = (h % 2) * DH
                nc.vector.tensor_mul(
                    out=AO[bi][h // 2][po:po + DH, sl], in0=ot[0:DH, sl],
                    in1=lbf[0:DH, sl])
                return
            # HW-validated chain: copy the psum l-row (partition 64) to SBUF,
            # DMA it to partition 0, reciprocal there, broadcast from
            # partition 0. Custom DVE ops and partition_broadcast only work
            # from partition 0 on real hardware; ACT can read psum anywhere.
            sl = slice(None) if half is None else slice(half * 512, (half + 1) * 512)
            w = n if half is None else 512
            lb = lrp.tile([DH + 1, n], f32, tag="lb", name="lb") if half != 1 \
                else emit_norm.lb_cur
            emit_norm.lb_cur = lb
            if norm_idx[0] % 2 < row_copy_act:
                nc.scalar.activation(lb[DH:DH + 1, sl], ot[DH:DH + 1, sl], Copy)
            else:
                nc.vector.tensor_copy(out=lb[DH:DH + 1, sl], in_=ot[DH:DH + 1, sl])
            norm_idx[0] += 1
            if recip_scatter and half is None:
                # scatter the l row across 128 partitions, reciprocal there
                # (128x less DVE row time), gather back to a partition-0 row.
                nsc = n // 128
                lr0s = lrp.tile([128, nsc], f32, tag="lr0s", bufs=1, name="lr0s")
                nc.sync.dma_start(out=lr0s[:], in_=lb[DH:DH + 1, :])
                lrrs = lrp.tile([128, nsc], f32, tag="lrrs", bufs=1, name="lrrs")
                nc.vector.reciprocal_approx_fast(out=lrrs[:], in_=lr0s[:])
                lrr = lrp.tile([1, n], f32, tag="lrr", bufs=1, name="lrr")
                nc.sync.dma_start(out=lrr[0:1, :], in_=lrrs[:])
            else:
                lr0 = lrp.tile([1, n], f32, tag="lr0", bufs=1, name="lr0") if half != 1 \
                    else emit_norm.lr0_cur
                emit_norm.lr0_cur = lr0
                nc.sync.dma_start(out=lr0[0:1, sl], in_=lb[DH:DH + 1, sl])
                lrr = lrp.tile([1, n], f32, tag="lrr", bufs=1, name="lrr") if half != 1 \
                    else emit_norm.lrr_cur
                emit_norm.lrr_cur = lrr
                nc.vector.reciprocal_approx_fast(out=lrr[0:1, sl], in_=lr0[0:1, sl])
            nc.gpsimd.partition_broadcast(lb[0:DH, sl], lrr[0:1, sl], channels=DH)
            if h % 2 == 0:
                nc.vector.tensor_mul(
                    out=AO[bi][h // 2][0:DH, sl], in0=ot[0:DH, sl], in1=lb[0:DH, sl])
            else:
                tmpo = lrp.tile([DH, n], f16, tag="tmpo", bufs=tmpo_bufs, name="tmpo") \
                    if half != 1 else emit_norm.tmpo_cur
                emit_norm.tmpo_cur = tmpo
                nc.vector.tensor_mul(out=tmpo[0:DH, sl], in0=ot[0:DH, sl], in1=lb[0:DH, sl])
                nc.sync.dma_start(out=AO[bi][h // 2][DH:128, sl], in_=tmpo[0:DH, sl])

        # ---- prologue compute: QK projections for ip=0 (heads 0,1).
        # The sim ring is empty this early, so borrow it: four chunks then
        # rotate through four psum slots instead of two.
        for bi in range(bpc):
            emit_qk_chunk("q", QT, bi, 0, ring="mm" if proq_mm else None,
                          first_split=(bi == 0 and bool(head_split)))
            emit_qk_chunk("k", KT, bi, 0, ring="mm" if proq_mm else None,
                          first_split=(bi == 0 and bool(head_split)))

        # ---- deadline-scheduled inject: chunk -> emission slot
        sched_ip1 = list(sched_ip1)
        sched = {}
        slots_v0 = [0, 2, 4, 6]
        slots_v1 = [8, 10, 12, 14]
        slots_ip = {1: sched_ip1, 2: [36, 44, 52, 60], 3: [68, 76, 84, 92]}
        if split_inj:
            for t2 in range(T2):
                sched[slots_v0[t2]] = [("vh", 0, t2, 0), ("vh", 0, t2, 1)]
                sched[slots_v1[t2]] = [("vh", 1, t2, 0), ("vh", 1, t2, 1)]
            for ip in range(1, KP):
                for i, (kind, bi) in enumerate((("q", 0), ("k", 0), ("q", 1), ("k", 1))):
                    sched[slots_ip[ip][i]] = [(kind + "h", bi, ip, 0),
                                              (kind + "h", bi, ip, 1)]
        else:
            for t2 in range(T2):
                sched[slots_v0[t2]] = [("v", 0, t2)]
                sched[slots_v1[t2]] = [("v", 1, t2)]
            for ip in range(1, KP):
                for i, (kind, bi) in enumerate((("q", 0), ("k", 0), ("q", 1), ("k", 1))):
                    sched[slots_ip[ip][i]] = [(kind, bi, ip)]
        for t2 in range(T2):
            sched[120 + 2 * t2] = [("f", 0, t2)]

        def emit_chunk(c):
            kind = c[0]
            if kind == "v":
                emit_v_chunk(c[1], c[2])
            elif kind == "vh":
                emit_v_chunk(c[1], c[2], blocks=(c[3],))
            elif kind == "q":
                emit_qk_chunk("q", QT, c[1], c[2])
            elif kind == "k":
                emit_qk_chunk("k", KT, c[1], c[2])
            elif kind == "qh":
                emit_qk_chunk("q", QT, c[1], c[2], halves=(c[3],))
            elif kind == "kh":
                emit_qk_chunk("k", KT, c[1], c[2], halves=(c[3],))
            elif kind == "f":
                emit_f_chunk(c[1], c[2])

        # ---- main attention loop; last head pair runs h-odd first so the
        # final stream is an even head (its norm-mul writes AO directly,
        # shortening the tail by one SBUF->SBUF DMA hop).
        stream_order = {}
        for hp in range(HP):
            h0, h1 = 2 * hp, 2 * hp + 1
            if hp == HP - 1:
                stream_order[hp] = [(h1, 0), (h1, 1), (h0, 0), (h0, 1)]
            else:
                stream_order[hp] = [(h0, 0), (h0, 1), (h1, 0), (h1, 1)]

        slot = 0
        for hp in range(HP):
            streams = stream_order[hp]
            first_h = streams[0][0]
            other_h = streams[2][0]
            for sidx, (h, bi) in enumerate(streams):
                is_last = (hp == HP - 1 and sidx == 3)
                ot_last = None
                if is_last:
                    ot_last = ps.tile([DH + 1, 1024], f32, tag="ot",
                                      bufs=ot_bufs, name="ot_last")
                et_row = []
                if inline_av_lag and not is_last:
                    start_av(h, bi, et_row, inline=True)
                sp_hoist = [None]
                for jt in range(NT):
                    # eb prefetch (jt pairs): sidx1 fetches this pair's other
                    # head, sidx2 fetches the next head-pair's first head.
                    if jt % 2 == 0:
                        if sidx == 1:
                            emit_eb_dma(other_h, jt // 2)
                        elif sidx == 2 and hp + 1 < HP:
                            emit_eb_dma(stream_order[hp + 1][0][0], jt // 2)
                    po = (h % 2) * 64

                    def emit_sim(j):
                        spj = ps.tile([128, 1024], f32, tag="mm", bufs=sim_bufs,
                                      name="spj")
                        for ihh in range(2):
                            nc.tensor.matmul(
                                spj[:, ihh * 512:(ihh + 1) * 512],
                                KT[bi][hp][po:po + 64, j * 128:(j + 1) * 128],
                                QT[bi][hp][po:po + 64, ihh * 512:(ihh + 1) * 512],
                                start=True, stop=True,
                            )
                        return spj

                    if jt == 1 and sp_hoist[0] is not None:
                        sp = sp_hoist[0]
                    else:
                        sp = emit_sim(jt)
                    if hoist_sim and jt == 0 and pend_any():
                        sp_hoist[0] = emit_sim(1)
                    eq = etp.tile([128, n], f16, tag="eq", bufs=8, name="eq")
                    nc.scalar.activation(eq[:], sp[:], Exp)
                    et = etp.tile([128, n], f16, tag="et", name="et")
                    use_pool = (not is_last) and jt in pool_jts
                    mul_eng = nc.gpsimd if use_pool else nc.vector
                    mul_eng.tensor_mul(out=et[:], in0=eq[:], in1=eb_tiles[(h, jt)])
                    et_row.append(et)
                    if is_last:
                        for ihh in range(2):
                            nc.tensor.matmul(
                                ot_last[:, ihh * 512:(ihh + 1) * 512],
                                VA[bi][jt][:, h * (DH + 1):(h + 1) * (DH + 1)],
                                et[:, ihh * 512:(ihh + 1) * 512],
                                start=(jt == 0), stop=(jt == NT - 1),
                            )
                    if jt == av_flush_slot and not inline_av_lag:
                        step_av(99)
                    if slot in sched:
                        for c in sched.pop(slot):
                            emit_chunk(c)
                    if inline_av_lag:
                        step_av(6)
                    slot += 1
                if not inline_av_lag:
                    step_av(99)  # flush any remaining AV of the previous stream
                if is_last:
                    if inline_av_lag:
                        step_av(99)  # drain all pending AV before the tail norm
                    if last_norm_split:
                        emit_norm(h, bi, ot_last, half=0)
                        emit_norm(h, bi, ot_last, half=1)
                    else:
                        emit_norm(h, bi, ot_last)
                elif not inline_av_lag:
                    start_av(h, bi, et_row)
        step_av(99)
        # drain: anything left, then F for batch 1
        for s in sorted(sched):
            for c in sched[s]:
                emit_chunk(c)
        for t2 in range(T2):
            emit_f_chunk(1, t2, ring=("mm" if (f_drain_mm and t2 % 2) else None),
                         fine=bool(drain_fine) and t2 == T2 - 1)

        for p in (ps, fop, lrp, etp, ebp, xtp, pers):
            p.release()

    nc.compile()
    return nc


def prep_inputs(x, Wq, Wk, Wv, rel_bias, Wo, bo, n=N, bpc=BPC, ncores=NCORES):
    """Host-side sharding/layout prep. Returns in_maps (one dict per core)."""
    f16 = np.float16
    x = np.asarray(x, dtype=np.float32)
    xT = np.ascontiguousarray(x.transpose(0, 2, 1)).astype(f16)   # [B, D, n]
    WqT = np.ascontiguousarray(Wq.T * np.float32(SCALE)).astype(f16)
    WkT = np.ascontiguousarray(Wk.T).astype(f16)
    WvT = np.ascontiguousarray(Wv.T).astype(f16)
    WoT = np.ascontiguousarray(Wo.T).astype(f16)
    expBT = np.ascontiguousarray(
        np.exp(np.asarray(rel_bias, dtype=np.float32).transpose(0, 2, 1))
    ).astype(f16)                                                  # [H, n(j), n(i)]
    in_maps = []
    for c in range(ncores):
        in_maps.append({
            "xT": np.ascontiguousarray(xT[c * bpc:(c + 1) * bpc]),
            "WqT": WqT, "WkT": WkT, "WvT": WvT, "WoT": WoT,
            "expBT": expBT,
        })
    return in_maps


_CACHE = {}


def kernel(x, Wq, Wk, Wv, rel_bias, Wo, bo):
    from concourse.bass_utils import run_bass_kernel_spmd

    if "nc" not in _CACHE:
        _CACHE["nc"] = build_nc(norm_fast=1, f_drain_mm=1)
    nc = _CACHE["nc"]
    in_maps = prep_inputs(x, Wq, Wk, Wv, rel_bias, Wo, bo)
    res = run_bass_kernel_spmd(nc, in_maps, core_ids=list(range(NCORES)))
    out = np.concatenate([res.results[c]["out"] for c in range(NCORES)], axis=0)
    out = out + np.asarray(bo, dtype=np.float32)[None, None, :]
    return np.ascontiguousarray(out, dtype=np.float32)



# revision 28
# speedup vs baseline: 1.0475x; 1.0326x over previous
"""CrossAttention kernel for 8x TRN2 NeuronCores (Bass/Tile), v2.

Reference computation (per batch b of 16, heads h=8, n=1024, d_model=512, dh=64):
    q = x @ Wq.T, k = x @ Wk.T, v = x @ Wv.T          (per-head slices)
    sim = q k^T * scale + rel_bias[h]
    attn = softmax(sim, axis=-1)
    out = (attn @ v) re-assembled over heads, then @ Wo.T + bo

Sharding: data-parallel over batch, 2 batches per core x 8 cores.

v2 design notes (vs v1 baseline at ~264us modeled):
  - all matmul operands fp16 (full-rate on PE at any tile size, better
    mantissa than bf16, halves weight/x/eb DMA vs f32).
  - softmax runs on transposed sim (j on partitions); rel_bias applied as
    exp(sim)*exp(bias^T) with exp(bias^T) precomputed on host in fp16;
    the multiply runs in-place on DVE in 16-bit 2x mode.
  - attn@V uses V in natural layout as lhsT with an appended ones column:
    the same matmul emits the softmax denominator l as psum row 64.
  - normalization (HW constraint: custom DVE ops and partition_broadcast
    only operate from partition 0, and only on SBUF): DVE-copy the psum
    l-row to SBUF, DMA it to partition 0, reciprocal_approx_fast there,
    gpsimd partition_broadcast, one 1024-wide DVE multiply. Odd heads land
    in AO rows 64..127 via one SBUF->SBUF DMA.
  - output is fp16 (host converts to f32 and adds bias bo) - halves the
    serial output-DMA drain at the end of the program.
  - PSUM split into two rings: "mm" (sim tiles only, so ACT never starves
    behind injected work) and "ot" (attn@V accumulators + all projection /
    output-projection chunks).
  - coarse DMAs: one descriptor-chain per weight matrix / x batch, rel-bias
    loaded in jt-pairs - halves serial HWDGE occupancy.
  - emission is software-pipelined: projections for ip0 first, then the
    attention stream loop with V/QK(ip1..3)/F chunks injected between
    sim slots so the PE never starves while ACT grinds exp.
"""

import numpy as np

HEADS = 8
DH = 64
B = 16
N = 1024
D = 512  # d_model = inner
SCALE = DH ** -0.5
NCORES = 8
BPC = B // NCORES  # batches per core


def build_nc(n=N, bpc=BPC, sim_bufs=2, ot_bufs=2, eb_bufs=7, et_bufs=18,
             lb_bufs=3, fo_bufs=6, tmpo_bufs=2, pool_mul_mod=0, row_copy_act=0, v_copy_act=0, last_norm_split=0, av_flush_slot=0, qk_copy_act_ip=-1, proq_mm=0, f_drain_mm=0, warmup_mms=10, x0_split=0, hoist_sim=1, recip_wide=0, split_inj=0, head_split=1, sched_ip1=(20, 24, 27, 30),
             f_copy_dve=0, recip_scatter=0, inline_av_lag=0, drain_fine=0,
             norm_fast=0, scr_dve=0, norm_delay=0,
             sched_ip2=(36, 44, 52, 60), sched_ip3=(68, 76, 84, 92),
             sched_f0=(120, 122, 124, 126), f0_fine=0, hp3_order=0,
             defer_qk_copy=0, pair_mul=0, drain_nt=0, eq_bufs=8, av_first=0):
    import concourse.mybir as mybir
    import concourse.tile as tile
    from concourse import bacc

    f32 = mybir.dt.float32
    f16 = mybir.dt.float16
    Exp = mybir.ActivationFunctionType.Exp
    Copy = mybir.ActivationFunctionType.Copy

    NT = n // 128            # n tiles of 128 (8)
    KP = D // 128            # d_model k-tiles (4)
    HP = HEADS // 2          # head pairs (4)
    T2 = NT // 2             # double-nt chunks (4)

    nc = bacc.Bacc(None, target_bir_lowering=False)

    xT_d = nc.dram_tensor("xT", [bpc, D, n], f16, kind="ExternalInput")
    wq_d = nc.dram_tensor("WqT", [D, D], f16, kind="ExternalInput")   # pre-scaled
    wk_d = nc.dram_tensor("WkT", [D, D], f16, kind="ExternalInput")
    wv_d = nc.dram_tensor("WvT", [D, D], f16, kind="ExternalInput")
    wo_d = nc.dram_tensor("WoT", [D, D], f16, kind="ExternalInput")
    eb_d = nc.dram_tensor("expBT", [HEADS, n, n], f16, kind="ExternalInput")
    out_d = nc.dram_tensor("out", [bpc, n, D], f16, kind="ExternalOutput")

    with tile.TileContext(nc) as tc:
        pers = tc.alloc_tile_pool(name="pers", bufs=1)
        # ---- persistent tiles
        QT = [[pers.tile([128, n], f16, tag=f"qt{bi}_{ip}", name=f"qt{bi}_{ip}")
               for ip in range(KP)] for bi in range(bpc)]
        KT = [[pers.tile([128, n], f16, tag=f"kt{bi}_{ip}", name=f"kt{bi}_{ip}")
               for ip in range(KP)] for bi in range(bpc)]
        VA = [[pers.tile([128, HEADS * (DH + 1)], f16, tag=f"va{bi}_{nt}",
                         name=f"va{bi}_{nt}") for nt in range(NT)]
              for bi in range(bpc)]
        AO = [[pers.tile([128, n], f16, tag=f"ao{bi}_{kp}", name=f"ao{bi}_{kp}")
               for kp in range(KP)] for bi in range(bpc)]
        # each weight matrix lives in one [128, KP, 512] tile (one DMA each)
        w4 = {}
        for wname, wd in (("q", wq_d), ("k", wk_d), ("v", wv_d), ("o", wo_d)):
            t = pers.tile([128, KP, D], f16, tag=f"w{wname}", name=f"w{wname}")
            w4[wname] = t
        xtp = tc.alloc_tile_pool(name="xt", bufs=1)
        xt = [xtp.tile([128, KP, n], f16, tag=f"x{bi}", name=f"x{bi}")
              for bi in range(bpc)]

        ebp = tc.alloc_tile_pool(name="eb", bufs=eb_bufs)
        etp = tc.alloc_tile_pool(name="et", bufs=et_bufs)
        lrp = tc.alloc_tile_pool(name="lr", bufs=lb_bufs)
        fop = tc.alloc_tile_pool(name="fop", bufs=1)
        ps = tc.alloc_tile_pool(name="ps", bufs=1, space="PSUM")

        # ---- prologue DMAs (one chained DMA per tensor)
        def dma_w(wname, wd):
            dst = w4[wname][:]
            src = wd.rearrange("(kp p) c -> p kp c", p=128)
            nc.sync.dma_start(out=dst, in_=src)

        def dma_x(bi, split=False):
            if split:
                for kp in range(KP):
                    nc.sync.dma_start(
                        out=xt[bi][:, kp, :],
                        in_=xT_d[bi, kp * 128:(kp + 1) * 128, :])
            else:
                nc.sync.dma_start(
                    out=xt[bi][:], in_=xT_d[bi].rearrange("(kp p) j -> p kp j", p=128))

        pool_jts = set(range(3, 3 + pool_mul_mod))
        av_jt_order = [j for j in range(NT) if j not in pool_jts] + sorted(pool_jts)
        eb_tiles = {}

        def emit_eb_dma(h, jp):
            """Load jt pair (2*jp, 2*jp+1) of head h as one [128, 2, n] tile."""
            t = ebp.tile([128, 2, n], f16, tag="eb", name="eb")
            nc.sync.dma_start(
                out=t[:],
                in_=eb_d[h, 2 * jp * 128:(2 * jp + 2) * 128, :].rearrange(
                    "(two p) i -> p two i", two=2))
            eb_tiles[(h, 2 * jp)] = t[:, 0, :]
            eb_tiles[(h, 2 * jp + 1)] = t[:, 1, :]

        dma_w("q", wq_d)
        dma_x(0, split=bool(x0_split))
        dma_w("k", wk_d)
        dma_x(1)
        dma_w("v", wv_d)
        dma_w("o", wo_d)
        for jp in range(NT // 2):
            emit_eb_dma(0, jp)

        # ---- PE warm-up: keep the tensor engine continuously busy through
        # the prologue DMAs so its p-state clock is fully ramped (213ns/row
        # instead of 427+) when the first real projection matmuls arrive.
        if warmup_mms:
            scr = pers.tile([128, 512], f16, tag="scr", name="scr")
            if scr_dve:
                nc.vector.memset(scr[:], 0.0)
            else:
                nc.gpsimd.memset(scr[:], 0.0)
            wps = ps.tile([128, 512], f32, tag="mm", bufs=sim_bufs, name="wps")
            for _ in range(warmup_mms):
                nc.tensor.matmul(wps[:], scr[:, 0:128], scr[:],
                                 start=True, stop=True)

        # ---- emitters -----------------------------------------------------
        inj_ring = [0]
        pend_copies = []

        def flush_copies():
            while pend_copies:
                pend_copies.pop(0)()

        def inj_tag():
            return "ot"

        def emit_qk_chunk(wname, DST, bi, ip, ring=None, halves=(0, 1), first_split=False):
            W_s = w4[wname]
            wide = len(halves) == 2 and not split_inj
            pt = ps.tile([128, 1024 if wide else 512], f32,
                         tag=ring or inj_tag(), bufs=ot_bufs, name="pt")
            for i, nh in enumerate(halves):
                base = i * 512 if wide else 0
                for kp in range(KP):
                    nc.tensor.matmul(
                        pt[:, base:base + 512],
                        W_s[:, kp, ip * 128:(ip + 1) * 128],
                        xt[bi][:, kp, nh * 512:(nh + 1) * 512],
                        start=(kp == 0), stop=(kp == KP - 1),
                    )
                if not wide:
                    nc.vector.tensor_copy(
                        out=DST[bi][ip][:, nh * 512:(nh + 1) * 512], in_=pt[:])
            if wide:
                if first_split:
                    # sim jt0 needs only K cols 0:128 / Q cols 0:512 - copy
                    # those first so the first sim fires sooner.
                    c0 = 128 if wname == "k" else 512
                    nc.vector.tensor_copy(out=DST[bi][ip][:, 0:c0], in_=pt[:, 0:c0])
                    nc.vector.tensor_copy(out=DST[bi][ip][:, c0:], in_=pt[:, c0:])
                elif defer_qk_copy and ip > 0:
                    pend_copies.append(
                        lambda d=DST[bi][ip], s=pt: nc.vector.tensor_copy(out=d[:], in_=s[:]))
                else:
                    nc.vector.tensor_copy(out=DST[bi][ip][:], in_=pt[:])

        def emit_v_chunk(bi, t2, blocks=(0, 1)):
            wide = len(blocks) == 2 and not split_inj
            pt = ps.tile([128, 1024 if wide else 512], f32, tag=inj_tag(),
                         bufs=ot_bufs, name="pt")
            for i, b in enumerate(blocks):
                nt = 2 * t2 + b
                base = i * 512 if wide else 0
                for kp in range(KP):
                    nc.tensor.matmul(
                        pt[:, base:base + 512],
                        xt[bi][:, kp, nt * 128:(nt + 1) * 128],
                        w4["v"][:, kp, :],
                        start=(kp == 0), stop=(kp == KP - 1),
                    )
                va = VA[bi][nt]
                nc.gpsimd.memset(va[:], 1.0)
                dst3 = va[:].rearrange("p (h c) -> p h c", c=DH + 1)[:, :, 0:DH]
                src3 = pt[:, base:base + 512].rearrange("p (h c) -> p h c", c=DH)
                nc.vector.tensor_copy(out=dst3, in_=src3)

        def emit_f_drain(pt, bi, nt, b):
            fo = fop.tile([128, 512], f16, tag="fo", bufs=fo_bufs, name="fo")
            if f_copy_dve:
                nc.vector.tensor_copy(out=fo[:], in_=pt[:, b * 512:(b + 1) * 512])
            else:
                nc.scalar.activation(fo[:], pt[:, b * 512:(b + 1) * 512], Copy)
            nc.sync.dma_start(out=out_d[bi, nt * 128:(nt + 1) * 128, :], in_=fo[:])

        def emit_f_chunk(bi, t2, ring=None, fine=False):
            pt = ps.tile([128, 1024], f32, tag=ring or inj_tag(), bufs=ot_bufs,
                         name="pt")
            for b in range(2):
                nt = 2 * t2 + b
                for kp in range(KP):
                    nc.tensor.matmul(
                        pt[:, b * 512:(b + 1) * 512],
                        AO[bi][kp][:, nt * 128:(nt + 1) * 128],
                        w4["o"][:, kp, :],
                        start=(kp == 0), stop=(kp == KP - 1),
                    )
                if fine:
                    emit_f_drain(pt, bi, nt, b)
            if not fine:
                for b in range(2):
                    emit_f_drain(pt, bi, 2 * t2 + b, b)

        def emit_f_nt(bi, nt, ring=None):
            pt = ps.tile([128, 512], f32, tag=ring or inj_tag(), bufs=ot_bufs, name="ptf")
            for kp in range(KP):
                nc.tensor.matmul(
                    pt[:, 0:512],
                    AO[bi][kp][:, nt * 128:(nt + 1) * 128],
                    w4["o"][:, kp, :],
                    start=(kp == 0), stop=(kp == KP - 1),
                )
            emit_f_drain(pt, bi, nt, 0)

        pend_avs = []
        pend_norms = []

        def pend_any():
            return bool(pend_avs)

        def queue_norm(h, bi, ot):
            if norm_delay and (h // 2) != HP - 1:
                pend_norms.append((h, bi, ot))
            else:
                flush_norms()
                emit_norm(h, bi, ot)

        def flush_norms():
            while pend_norms:
                emit_norm(*pend_norms.pop(0))

        def start_av(h, bi, et_row, inline=False):
            ot = ps.tile([DH + 1, 1024], f32, tag="ot", bufs=ot_bufs, name="ot")
            if inline:
                mms = [(ihh, jt) for jt in av_jt_order for ihh in range(2)]
            else:
                mms = [(ihh, jt) for ihh in range(2) for jt in av_jt_order]
            pend_avs.append({"h": h, "bi": bi, "et": et_row, "ot": ot,
                             "mms": mms, "lag": inline_av_lag if inline else 0})

        def step_av(nmm):
            while nmm > 0 and pend_avs:
                st = pend_avs[0]
                h, bi, et_row, ot = st["h"], st["bi"], st["et"], st["ot"]
                if not st["mms"]:
                    pend_avs.pop(0)
                    queue_norm(h, bi, ot)
                    continue
                ihh, jt = st["mms"][0]
                lag = st["lag"] if st is pend_avs[-1] else 0
                if jt >= len(et_row) - lag:
                    return
                st["mms"].pop(0)
                nc.tensor.matmul(
                    ot[:, ihh * 512:(ihh + 1) * 512],
                    VA[bi][jt][:, h * (DH + 1):(h + 1) * (DH + 1)],
                    et_row[jt][:, ihh * 512:(ihh + 1) * 512],
                    start=(jt == av_jt_order[0]), stop=(jt == av_jt_order[-1]),
                )
                nmm -= 1
                if not st["mms"]:
                    pend_avs.pop(0)
                    queue_norm(h, bi, ot)

        norm_idx = [0]

        def emit_norm(h, bi, ot, half=None):
            if norm_fast:
                # Short chain: cross-partition-base DVE copy moves the psum
                # l-row (partition 64) straight to a partition-0 SBUF row (no
                # DMA hop), reciprocal there, broadcast, and the apply writes
                # odd-head AO rows at partition base 64 directly (no tmpo DMA).
                sl = slice(None) if half is None else slice(half * 512, (half + 1) * 512)
                lbf = lrp.tile([DH, n], f32, tag="lb", name="lbf") if half != 1 \
                    else emit_norm.lb_cur
                emit_norm.lb_cur = lbf
                lrow = lrp.tile([1, n], f32, tag="lrow", bufs=2, name="lrow") if half != 1 \
                    else emit_norm.lrow_cur
                emit_norm.lrow_cur = lrow
                nc.vector.tensor_copy(out=lrow[0:1, sl], in_=ot[DH:DH + 1, sl])
                if recip_scatter and half is None:
                    nsc = n // 128
                    lr0s = lrp.tile([128, nsc], f32, tag="lr0s", bufs=2, name="lr0s")
                    nc.sync.dma_start(out=lr0s[:], in_=lrow[0:1, :])
                    nc.vector.reciprocal_approx_fast(out=lr0s[:], in_=lr0s[:])
                    nc.sync.dma_start(out=lrow[0:1, :], in_=lr0s[:])
                else:
                    nc.vector.reciprocal_approx_fast(out=lrow[0:1, sl], in_=lrow[0:1, sl])
                nc.gpsimd.partition_broadcast(lbf[0:DH, sl], lrow[0:1, sl], channels=DH)
                po = (h % 2) * DH
                nc.vector.tensor_mul(
                    out=AO[bi][h // 2][po:po + DH, sl], in0=ot[0:DH, sl],
                    in1=lbf[0:DH, sl])
                return
            # HW-validated chain: copy the psum l-row (partition 64) to SBUF,
            # DMA it to partition 0, reciprocal there, broadcast from
            # partition 0. Custom DVE ops and partition_broadcast only work
            # from partition 0 on real hardware; ACT can read psum anywhere.
            sl = slice(None) if half is None else slice(half * 512, (half + 1) * 512)
            w = n if half is None else 512
            lb = lrp.tile([DH + 1, n], f32, tag="lb", name="lb") if half != 1 \
                else emit_norm.lb_cur
            emit_norm.lb_cur = lb
            if norm_idx[0] % 2 < row_copy_act:
                nc.scalar.activation(lb[DH:DH + 1, sl], ot[DH:DH + 1, sl], Copy)
            else:
                nc.vector.tensor_copy(out=lb[DH:DH + 1, sl], in_=ot[DH:DH + 1, sl])
            norm_idx[0] += 1
            if recip_scatter and half is None:
                # scatter the l row across 128 partitions, reciprocal there
                # (128x less DVE row time), gather back to a partition-0 row.
                nsc = n // 128
                lr0s = lrp.tile([128, nsc], f32, tag="lr0s", bufs=1, name="lr0s")
                nc.sync.dma_start(out=lr0s[:], in_=lb[DH:DH + 1, :])
                lrrs = lrp.tile([128, nsc], f32, tag="lrrs", bufs=1, name="lrrs")
                nc.vector.reciprocal_approx_fast(out=lrrs[:], in_=lr0s[:])
                lrr = lrp.tile([1, n], f32, tag="lrr", bufs=1, name="lrr")
                nc.sync.dma_start(out=lrr[0:1, :], in_=lrrs[:])
            else:
                lr0 = lrp.tile([1, n], f32, tag="lr0", bufs=1, name="lr0") if half != 1 \
                    else emit_norm.lr0_cur
                emit_norm.lr0_cur = lr0
                nc.sync.dma_start(out=lr0[0:1, sl], in_=lb[DH:DH + 1, sl])
                lrr = lrp.tile([1, n], f32, tag="lrr", bufs=1, name="lrr") if half != 1 \
                    else emit_norm.lrr_cur
                emit_norm.lrr_cur = lrr
                nc.vector.reciprocal_approx_fast(out=lrr[0:1, sl], in_=lr0[0:1, sl])
            nc.gpsimd.partition_broadcast(lb[0:DH, sl], lrr[0:1, sl], channels=DH)
            if h % 2 == 0:
                nc.vector.tensor_mul(
                    out=AO[bi][h // 2][0:DH, sl], in0=ot[0:DH, sl], in1=lb[0:DH, sl])
            else:
                tmpo = lrp.tile([DH, n], f16, tag="tmpo", bufs=tmpo_bufs, name="tmpo") \
                    if half != 1 else emit_norm.tmpo_cur
                emit_norm.tmpo_cur = tmpo
                nc.vector.tensor_mul(out=tmpo[0:DH, sl], in0=ot[0:DH, sl], in1=lb[0:DH, sl])
                nc.sync.dma_start(out=AO[bi][h // 2][DH:128, sl], in_=tmpo[0:DH, sl])

        # ---- prologue compute: QK projections for ip=0 (heads 0,1).
        # The sim ring is empty this early, so borrow it: four chunks then
        # rotate through four psum slots instead of two.
        for bi in range(bpc):
            emit_qk_chunk("q", QT, bi, 0, ring="mm" if proq_mm else None,
                          first_split=(bi == 0 and bool(head_split)))
            emit_qk_chunk("k", KT, bi, 0, ring="mm" if proq_mm else None,
                          first_split=(bi == 0 and bool(head_split)))

        # ---- deadline-scheduled inject: chunk -> emission slot
        sched_ip1 = list(sched_ip1)
        sched = {}
        slots_v0 = [0, 2, 4, 6]
        slots_v1 = [8, 10, 12, 14]
        slots_ip = {1: sched_ip1, 2: list(sched_ip2), 3: list(sched_ip3)}
        if split_inj:
            for t2 in range(T2):
                sched[slots_v0[t2]] = [("vh", 0, t2, 0), ("vh", 0, t2, 1)]
                sched[slots_v1[t2]] = [("vh", 1, t2, 0), ("vh", 1, t2, 1)]
            for ip in range(1, KP):
                for i, (kind, bi) in enumerate((("q", 0), ("k", 0), ("q", 1), ("k", 1))):
                    sched[slots_ip[ip][i]] = [(kind + "h", bi, ip, 0),
                                              (kind + "h", bi, ip, 1)]
        else:
            for t2 in range(T2):
                sched[slots_v0[t2]] = [("v", 0, t2)]
                sched[slots_v1[t2]] = [("v", 1, t2)]
            for ip in range(1, KP):
                for i, (kind, bi) in enumerate((("q", 0), ("k", 0), ("q", 1), ("k", 1))):
                    sched[slots_ip[ip][i]] = [(kind, bi, ip)]
        if f0_fine:
            for i, s in enumerate(sched_f0):
                sched.setdefault(s, []).append(("ff", 0, i))
        else:
            for i, t2 in enumerate(range(T2)):
                sched.setdefault(sched_f0[i], []).append(("f", 0, t2))

        def emit_chunk(c):
            kind = c[0]
            if kind == "v":
                emit_v_chunk(c[1], c[2])
            elif kind == "vh":
                emit_v_chunk(c[1], c[2], blocks=(c[3],))
            elif kind == "q":
                emit_qk_chunk("q", QT, c[1], c[2])
            elif kind == "k":
                emit_qk_chunk("k", KT, c[1], c[2])
            elif kind == "qh":
                emit_qk_chunk("q", QT, c[1], c[2], halves=(c[3],))
            elif kind == "kh":
                emit_qk_chunk("k", KT, c[1], c[2], halves=(c[3],))
            elif kind == "f":
                flush_norms()
                emit_f_chunk(c[1], c[2])
            elif kind == "ff":
                flush_norms()
                emit_f_nt(c[1], c[2])

        # ---- main attention loop; last head pair runs h-odd first so the
        # final stream is an even head (its norm-mul writes AO directly,
        # shortening the tail by one SBUF->SBUF DMA hop).
        stream_order = {}
        for hp in range(HP):
            h0, h1 = 2 * hp, 2 * hp + 1
            if hp == HP - 1:
                if hp3_order:
                    stream_order[hp] = [(h1, 0), (h0, 0), (h1, 1), (h0, 1)]
                else:
                    stream_order[hp] = [(h1, 0), (h1, 1), (h0, 0), (h0, 1)]
            else:
                stream_order[hp] = [(h0, 0), (h0, 1), (h1, 0), (h1, 1)]

        slot = 0
        for hp in range(HP):
            streams = stream_order[hp]
            first_h = streams[0][0]
            other_h = streams[2][0]
            for sidx, (h, bi) in enumerate(streams):
                is_last = (hp == HP - 1 and sidx == 3)
                ot_last = None
                if is_last:
                    ot_last = ps.tile([DH + 1, 1024], f32, tag="ot",
                                      bufs=ot_bufs, name="ot_last")
                et_row = []
                if inline_av_lag and not is_last:
                    start_av(h, bi, et_row, inline=True)
                sp_hoist = [None]
                for jt in range(NT):
                    # eb prefetch (jt pairs): sidx1 fetches this pair's other
                    # head, sidx2 fetches the next head-pair's first head.
                    if jt % 2 == 0:
                        if hp3_order and hp == HP - 1:
                            if sidx == 0:
                                emit_eb_dma(streams[1][0], jt // 2)
                        elif sidx == 1:
                            emit_eb_dma(other_h, jt // 2)
                        if sidx == 2 and hp + 1 < HP:
                            if hp3_order and hp + 1 == HP - 1:
                                pass
                            emit_eb_dma(stream_order[hp + 1][0][0], jt // 2)
                    po = (h % 2) * 64

                    def emit_sim(j):
                        spj = ps.tile([128, 1024], f32, tag="mm", bufs=sim_bufs,
                                      name="spj")
                        for ihh in range(2):
                            nc.tensor.matmul(
                                spj[:, ihh * 512:(ihh + 1) * 512],
                                KT[bi][hp][po:po + 64, j * 128:(j + 1) * 128],
                                QT[bi][hp][po:po + 64, ihh * 512:(ihh + 1) * 512],
                                start=True, stop=True,
                            )
                        return spj

                    if av_first and jt <= 1 and not inline_av_lag:
                        # ready av matmuls of the previous stream go ahead of
                        # the boundary sims (which wait on ACT freeing the
                        # sim psum ring) to avoid in-order head blocking.
                        step_av(av_first)
                    if jt == 1 and sp_hoist[0] is not None:
                        sp = sp_hoist[0]
                    else:
                        sp = emit_sim(jt)
                    if hoist_sim and jt == 0 and pend_any():
                        sp_hoist[0] = emit_sim(1)
                    eq = etp.tile([128, n], f16, tag="eq", bufs=eq_bufs, name="eq")
                    nc.scalar.activation(eq[:], sp[:], Exp)
                    et = etp.tile([128, n], f16, tag="et", name="et")
                    use_pool = (not is_last) and jt in pool_jts
                    mul_eng = nc.gpsimd if use_pool else nc.vector
                    mul_eng.tensor_mul(out=et[:], in0=eq[:], in1=eb_tiles[(h, jt)])
                    et_row.append(et)
                    if is_last:
                        for ihh in range(2):
                            nc.tensor.matmul(
                                ot_last[:, ihh * 512:(ihh + 1) * 512],
                                VA[bi][jt][:, h * (DH + 1):(h + 1) * (DH + 1)],
                                et[:, ihh * 512:(ihh + 1) * 512],
                                start=(jt == 0), stop=(jt == NT - 1),
                            )
                    flush_copies()
                    if jt == av_flush_slot and not inline_av_lag:
                        step_av(99)
                    if slot in sched:
                        for c in sched.pop(slot):
                            emit_chunk(c)
                    if inline_av_lag:
                        step_av(6)
                    if norm_delay and jt == norm_delay:
                        flush_norms()
                    slot += 1
                if not inline_av_lag:
                    step_av(99)  # flush any remaining AV of the previous stream
                if is_last:
                    if inline_av_lag:
                        step_av(99)  # drain all pending AV before the tail norm
                    flush_norms()
                    if last_norm_split:
                        emit_norm(h, bi, ot_last, half=0)
                        emit_norm(h, bi, ot_last, half=1)
                    else:
                        emit_norm(h, bi, ot_last)
                elif not inline_av_lag:
                    start_av(h, bi, et_row)
        step_av(99)
        flush_norms()
        # drain: anything left, then F for batch 1
        for s in sorted(sched):
            for c in sched[s]:
                emit_chunk(c)
        if drain_nt:
            for nt in range(NT):
                emit_f_nt(1, nt, ring=("mm" if (f_drain_mm and nt % 2) else None))
        else:
            for t2 in range(T2):
                emit_f_chunk(1, t2, ring=("mm" if (f_drain_mm and t2 % 2) else None),
                             fine=bool(drain_fine) and t2 == T2 - 1)

        for p in (ps, fop, lrp, etp, ebp, xtp, pers):
            p.release()

    nc.compile()
    return nc


def prep_inputs(x, Wq, Wk, Wv, rel_bias, Wo, bo, n=N, bpc=BPC, ncores=NCORES):
    """Host-side sharding/layout prep. Returns in_maps (one dict per core)."""
    f16 = np.float16
    x = np.asarray(x, dtype=np.float32)
    xT = np.ascontiguousarray(x.transpose(0, 2, 1)).astype(f16)   # [B, D, n]
    WqT = np.ascontiguousarray(Wq.T * np.float32(SCALE)).astype(f16)
    WkT = np.ascontiguousarray(Wk.T).astype(f16)
    WvT = np.ascontiguousarray(Wv.T).astype(f16)
    WoT = np.ascontiguousarray(Wo.T).astype(f16)
    expBT = np.ascontiguousarray(
        np.exp(np.asarray(rel_bias, dtype=np.float32).transpose(0, 2, 1))
    ).astype(f16)                                                  # [H, n(j), n(i)]
    in_maps = []
    for c in range(ncores):
        in_maps.append({
            "xT": np.ascontiguousarray(xT[c * bpc:(c + 1) * bpc]),
            "WqT": WqT, "WkT": WkT, "WvT": WvT, "WoT": WoT,
            "expBT": expBT,
        })
    return in_maps


BEST_KW = dict(norm_fast=1, f_drain_mm=1, norm_delay=6, hp3_order=1,
               f0_fine=1, sched_f0=(113, 114, 115, 117, 119, 121, 123, 125),
               eb_bufs=8, drain_nt=1, eq_bufs=10, et_bufs=16, last_norm_split=1)
_CACHE = {}


def kernel(x, Wq, Wk, Wv, rel_bias, Wo, bo):
    from concourse.bass_utils import run_bass_kernel_spmd

    if "nc" not in _CACHE:
        _CACHE["nc"] = build_nc(**BEST_KW)
    nc = _CACHE["nc"]
    in_maps = prep_inputs(x, Wq, Wk, Wv, rel_bias, Wo, bo)
    res = run_bass_kernel_spmd(nc, in_maps, core_ids=list(range(NCORES)))
    out = np.concatenate([res.results[c]["out"] for c in range(NCORES)], axis=0)
    out = out + np.asarray(bo, dtype=np.float32)[None, None, :]
    return np.ascontiguousarray(out, dtype=np.float32)



# revision 32
# speedup vs baseline: 1.0491x; 1.0016x over previous
"""CrossAttention kernel for 8x TRN2 NeuronCores (Bass/Tile), v2.

Reference computation (per batch b of 16, heads h=8, n=1024, d_model=512, dh=64):
    q = x @ Wq.T, k = x @ Wk.T, v = x @ Wv.T          (per-head slices)
    sim = q k^T * scale + rel_bias[h]
    attn = softmax(sim, axis=-1)
    out = (attn @ v) re-assembled over heads, then @ Wo.T + bo

Sharding: data-parallel over batch, 2 batches per core x 8 cores.

v2 design notes (vs v1 baseline at ~264us modeled):
  - all matmul operands fp16 (full-rate on PE at any tile size, better
    mantissa than bf16, halves weight/x/eb DMA vs f32).
  - softmax runs on transposed sim (j on partitions); rel_bias applied as
    exp(sim)*exp(bias^T) with exp(bias^T) precomputed on host in fp16;
    the multiply runs in-place on DVE in 16-bit 2x mode.
  - attn@V uses V in natural layout as lhsT with an appended ones column:
    the same matmul emits the softmax denominator l as psum row 64.
  - normalization (HW constraint: custom DVE ops and partition_broadcast
    only operate from partition 0, and only on SBUF): DVE-copy the psum
    l-row to SBUF, DMA it to partition 0, reciprocal_approx_fast there,
    gpsimd partition_broadcast, one 1024-wide DVE multiply. Odd heads land
    in AO rows 64..127 via one SBUF->SBUF DMA.
  - output is fp16 (host converts to f32 and adds bias bo) - halves the
    serial output-DMA drain at the end of the program.
  - PSUM split into two rings: "mm" (sim tiles only, so ACT never starves
    behind injected work) and "ot" (attn@V accumulators + all projection /
    output-projection chunks).
  - coarse DMAs: one descriptor-chain per weight matrix / x batch, rel-bias
    loaded in jt-pairs - halves serial HWDGE occupancy.
  - emission is software-pipelined: projections for ip0 first, then the
    attention stream loop with V/QK(ip1..3)/F chunks injected between
    sim slots so the PE never starves while ACT grinds exp.
"""

import numpy as np

HEADS = 8
DH = 64
B = 16
N = 1024
D = 512  # d_model = inner
SCALE = DH ** -0.5
NCORES = 8
BPC = B // NCORES  # batches per core


def build_nc(n=N, bpc=BPC, sim_bufs=2, ot_bufs=2, eb_bufs=7, et_bufs=18,
             lb_bufs=3, fo_bufs=6, tmpo_bufs=2, pool_mul_mod=0, row_copy_act=0, v_copy_act=0, last_norm_split=0, av_flush_slot=0, qk_copy_act_ip=-1, proq_mm=0, f_drain_mm=0, warmup_mms=10, x0_split=0, hoist_sim=1, recip_wide=0, split_inj=0, head_split=1, sched_ip1=(20, 24, 27, 30),
             f_copy_dve=0, recip_scatter=0, inline_av_lag=0, drain_fine=0,
             norm_fast=0, scr_dve=0, norm_delay=0,
             sched_ip2=(36, 44, 52, 60), sched_ip3=(68, 76, 84, 92),
             sched_f0=(120, 122, 124, 126), f0_fine=0, hp3_order=0,
             defer_qk_copy=0, pair_mul=0, drain_nt=0, eq_bufs=8, av_first=0,
             sim7_ot=0, split_tiles=0):
    import concourse.mybir as mybir
    import concourse.tile as tile
    from concourse import bacc

    f32 = mybir.dt.float32
    f16 = mybir.dt.float16
    Exp = mybir.ActivationFunctionType.Exp
    Copy = mybir.ActivationFunctionType.Copy

    NT = n // 128            # n tiles of 128 (8)
    KP = D // 128            # d_model k-tiles (4)
    HP = HEADS // 2          # head pairs (4)
    T2 = NT // 2             # double-nt chunks (4)

    nc = bacc.Bacc(None, target_bir_lowering=False)

    xT_d = nc.dram_tensor("xT", [bpc, D, n], f16, kind="ExternalInput")
    wq_d = nc.dram_tensor("WqT", [D, D], f16, kind="ExternalInput")   # pre-scaled
    wk_d = nc.dram_tensor("WkT", [D, D], f16, kind="ExternalInput")
    wv_d = nc.dram_tensor("WvT", [D, D], f16, kind="ExternalInput")
    wo_d = nc.dram_tensor("WoT", [D, D], f16, kind="ExternalInput")
    eb_d = nc.dram_tensor("expBT", [HEADS, n, n], f16, kind="ExternalInput")
    out_d = nc.dram_tensor("out", [bpc, n, D], f16, kind="ExternalOutput")

    with tile.TileContext(nc) as tc:
        pers = tc.alloc_tile_pool(name="pers", bufs=1)
        # ---- persistent tiles
        QT = [[pers.tile([128, n], f16, tag=f"qt{bi}_{ip}", name=f"qt{bi}_{ip}")
               for ip in range(KP)] for bi in range(bpc)]
        KT = [[pers.tile([128, n], f16, tag=f"kt{bi}_{ip}", name=f"kt{bi}_{ip}")
               for ip in range(KP)] for bi in range(bpc)]
        VA = [[pers.tile([128, HEADS * (DH + 1)], f16, tag=f"va{bi}_{nt}",
                         name=f"va{bi}_{nt}") for nt in range(NT)]
              for bi in range(bpc)]
        AO = [[pers.tile([128, n], f16, tag=f"ao{bi}_{kp}", name=f"ao{bi}_{kp}")
               for kp in range(KP)] for bi in range(bpc)]
        # each weight matrix lives in one [128, KP, 512] tile (one DMA each)
        # (or per-kp tiles when split_tiles, so first-chunk matmuls can start
        # as soon as their kp slice of W and x has landed)
        w4 = {}
        for wname, wd in (("q", wq_d), ("k", wk_d), ("v", wv_d), ("o", wo_d)):
            if split_tiles:
                w4[wname] = [pers.tile([128, D], f16, tag=f"w{wname}{kp}",
                                       name=f"w{wname}{kp}") for kp in range(KP)]
            else:
                w4[wname] = pers.tile([128, KP, D], f16, tag=f"w{wname}",
                                      name=f"w{wname}")
        xtp = tc.alloc_tile_pool(name="xt", bufs=1)
        if split_tiles:
            xt = [[xtp.tile([128, n], f16, tag=f"x{bi}_{kp}", name=f"x{bi}_{kp}")
                   for kp in range(KP)] for bi in range(bpc)]
        else:
            xt = [xtp.tile([128, KP, n], f16, tag=f"x{bi}", name=f"x{bi}")
                  for bi in range(bpc)]

        def w_ap(wname, kp, sl=slice(None)):
            if split_tiles:
                return w4[wname][kp][:, sl]
            return w4[wname][:, kp, sl]

        def x_ap(bi, kp, sl=slice(None)):
            if split_tiles:
                return xt[bi][kp][:, sl]
            return xt[bi][:, kp, sl]

        ebp = tc.alloc_tile_pool(name="eb", bufs=eb_bufs)
        etp = tc.alloc_tile_pool(name="et", bufs=et_bufs)
        lrp = tc.alloc_tile_pool(name="lr", bufs=lb_bufs)
        fop = tc.alloc_tile_pool(name="fop", bufs=1)
        ps = tc.alloc_tile_pool(name="ps", bufs=1, space="PSUM")

        # ---- prologue DMAs (one chained DMA per tensor)
        def dma_w(wname, wd, kp=None):
            if split_tiles:
                for k in ([kp] if kp is not None else range(KP)):
                    nc.sync.dma_start(out=w4[wname][k][:],
                                      in_=wd[k * 128:(k + 1) * 128, :])
                return
            dst = w4[wname][:]
            src = wd.rearrange("(kp p) c -> p kp c", p=128)
            nc.sync.dma_start(out=dst, in_=src)

        def dma_x(bi, split=False, kp=None):
            if split_tiles:
                for k in ([kp] if kp is not None else range(KP)):
                    nc.sync.dma_start(out=xt[bi][k][:],
                                      in_=xT_d[bi, k * 128:(k + 1) * 128, :])
                return
            if split:
                for kp_ in range(KP):
                    nc.sync.dma_start(
                        out=xt[bi][:, kp_, :],
                        in_=xT_d[bi, kp_ * 128:(kp_ + 1) * 128, :])
            else:
                nc.sync.dma_start(
                    out=xt[bi][:], in_=xT_d[bi].rearrange("(kp p) j -> p kp j", p=128))

        pool_jts = set(range(3, 3 + pool_mul_mod))
        av_jt_order = [j for j in range(NT) if j not in pool_jts] + sorted(pool_jts)
        eb_tiles = {}
        eb_pairs = {}

        def emit_eb_dma(h, jp):
            """Load jt pair (2*jp, 2*jp+1) of head h as one [128, 2, n] tile."""
            t = ebp.tile([128, 2, n], f16, tag="eb", name="eb")
            nc.sync.dma_start(
                out=t[:],
                in_=eb_d[h, 2 * jp * 128:(2 * jp + 2) * 128, :].rearrange(
                    "(two p) i -> p two i", two=2))
            eb_tiles[(h, 2 * jp)] = t[:, 0, :]
            eb_tiles[(h, 2 * jp + 1)] = t[:, 1, :]
            eb_pairs[(h, jp)] = t[:]

        if split_tiles:
            for kp in range(KP):
                dma_w("q", wq_d, kp=kp)
                dma_x(0, kp=kp)
            dma_w("k", wk_d)
            dma_x(1)
            dma_w("v", wv_d)
            dma_w("o", wo_d)
        else:
            dma_w("q", wq_d)
            dma_x(0, split=bool(x0_split))
            dma_w("k", wk_d)
            dma_x(1)
            dma_w("v", wv_d)
            dma_w("o", wo_d)
        for jp in range(NT // 2):
            emit_eb_dma(0, jp)

        # ---- PE warm-up: keep the tensor engine continuously busy through
        # the prologue DMAs so its p-state clock is fully ramped (213ns/row
        # instead of 427+) when the first real projection matmuls arrive.
        if warmup_mms:
            scr = pers.tile([128, 512], f16, tag="scr", name="scr")
            if scr_dve:
                nc.vector.memset(scr[:], 0.0)
            else:
                nc.gpsimd.memset(scr[:], 0.0)
            wps = ps.tile([128, 512], f32, tag="mm", bufs=sim_bufs, name="wps")
            for _ in range(warmup_mms):
                nc.tensor.matmul(wps[:], scr[:, 0:128], scr[:],
                                 start=True, stop=True)

        # ---- emitters -----------------------------------------------------
        inj_ring = [0]
        pend_copies = []

        def flush_copies():
            while pend_copies:
                pend_copies.pop(0)()

        def inj_tag():
            return "ot"

        def emit_qk_chunk(wname, DST, bi, ip, ring=None, halves=(0, 1), first_split=False):
            W_s = w4[wname]
            wide = len(halves) == 2 and not split_inj
            pt = ps.tile([128, 1024 if wide else 512], f32,
                         tag=ring or inj_tag(), bufs=ot_bufs, name="pt")
            for i, nh in enumerate(halves):
                base = i * 512 if wide else 0
                for kp in range(KP):
                    nc.tensor.matmul(
                        pt[:, base:base + 512],
                        w_ap(wname, kp, slice(ip * 128, (ip + 1) * 128)),
                        x_ap(bi, kp, slice(nh * 512, (nh + 1) * 512)),
                        start=(kp == 0), stop=(kp == KP - 1),
                    )
                if not wide:
                    nc.vector.tensor_copy(
                        out=DST[bi][ip][:, nh * 512:(nh + 1) * 512], in_=pt[:])
            if wide:
                if first_split:
                    # sim jt0 needs only K cols 0:128 / Q cols 0:512 - copy
                    # those first so the first sim fires sooner.
                    c0 = 128 if wname == "k" else 512
                    nc.vector.tensor_copy(out=DST[bi][ip][:, 0:c0], in_=pt[:, 0:c0])
                    nc.vector.tensor_copy(out=DST[bi][ip][:, c0:], in_=pt[:, c0:])
                elif defer_qk_copy and ip > 0:
                    pend_copies.append(
                        lambda d=DST[bi][ip], s=pt: nc.vector.tensor_copy(out=d[:], in_=s[:]))
                else:
                    nc.vector.tensor_copy(out=DST[bi][ip][:], in_=pt[:])

        def emit_v_chunk(bi, t2, blocks=(0, 1)):
            wide = len(blocks) == 2 and not split_inj
            pt = ps.tile([128, 1024 if wide else 512], f32, tag=inj_tag(),
                         bufs=ot_bufs, name="pt")
            for i, b in enumerate(blocks):
                nt = 2 * t2 + b
                base = i * 512 if wide else 0
                for kp in range(KP):
                    nc.tensor.matmul(
                        pt[:, base:base + 512],
                        x_ap(bi, kp, slice(nt * 128, (nt + 1) * 128)),
                        w_ap("v", kp),
                        start=(kp == 0), stop=(kp == KP - 1),
                    )
                va = VA[bi][nt]
                nc.gpsimd.memset(va[:], 1.0)
                dst3 = va[:].rearrange("p (h c) -> p h c", c=DH + 1)[:, :, 0:DH]
                src3 = pt[:, base:base + 512].rearrange("p (h c) -> p h c", c=DH)
                nc.vector.tensor_copy(out=dst3, in_=src3)

        def emit_f_drain(pt, bi, nt, b):
            fo = fop.tile([128, 512], f16, tag="fo", bufs=fo_bufs, name="fo")
            if f_copy_dve:
                nc.vector.tensor_copy(out=fo[:], in_=pt[:, b * 512:(b + 1) * 512])
            else:
                nc.scalar.activation(fo[:], pt[:, b * 512:(b + 1) * 512], Copy)
            nc.sync.dma_start(out=out_d[bi, nt * 128:(nt + 1) * 128, :], in_=fo[:])

        def emit_f_chunk(bi, t2, ring=None, fine=False):
            pt = ps.tile([128, 1024], f32, tag=ring or inj_tag(), bufs=ot_bufs,
                         name="pt")
            for b in range(2):
                nt = 2 * t2 + b
                for kp in range(KP):
                    nc.tensor.matmul(
                        pt[:, b * 512:(b + 1) * 512],
                        AO[bi][kp][:, nt * 128:(nt + 1) * 128],
                        w_ap("o", kp),
                        start=(kp == 0), stop=(kp == KP - 1),
                    )
                if fine:
                    emit_f_drain(pt, bi, nt, b)
            if not fine:
                for b in range(2):
                    emit_f_drain(pt, bi, 2 * t2 + b, b)

        def emit_f_nt(bi, nt, ring=None):
            pt = ps.tile([128, 512], f32, tag=ring or inj_tag(), bufs=ot_bufs, name="ptf")
            for kp in range(KP):
                nc.tensor.matmul(
                    pt[:, 0:512],
                    AO[bi][kp][:, nt * 128:(nt + 1) * 128],
                    w_ap("o", kp),
                    start=(kp == 0), stop=(kp == KP - 1),
                )
            emit_f_drain(pt, bi, nt, 0)

        pend_avs = []
        pend_norms = []

        def pend_any():
            return bool(pend_avs)

        def queue_norm(h, bi, ot):
            if norm_delay and (h // 2) != HP - 1:
                pend_norms.append((h, bi, ot))
            else:
                flush_norms()
                emit_norm(h, bi, ot)

        def flush_norms():
            while pend_norms:
                emit_norm(*pend_norms.pop(0))

        def start_av(h, bi, et_row, inline=False):
            ot = ps.tile([DH + 1, 1024], f32, tag="ot", bufs=ot_bufs, name="ot")
            if inline:
                mms = [(ihh, jt) for jt in av_jt_order for ihh in range(2)]
            else:
                mms = [(ihh, jt) for ihh in range(2) for jt in av_jt_order]
            pend_avs.append({"h": h, "bi": bi, "et": et_row, "ot": ot,
                             "mms": mms, "lag": inline_av_lag if inline else 0})

        def step_av(nmm):
            while nmm > 0 and pend_avs:
                st = pend_avs[0]
                h, bi, et_row, ot = st["h"], st["bi"], st["et"], st["ot"]
                if not st["mms"]:
                    pend_avs.pop(0)
                    queue_norm(h, bi, ot)
                    continue
                ihh, jt = st["mms"][0]
                lag = st["lag"] if st is pend_avs[-1] else 0
                if jt >= len(et_row) - lag:
                    return
                st["mms"].pop(0)
                nc.tensor.matmul(
                    ot[:, ihh * 512:(ihh + 1) * 512],
                    VA[bi][jt][:, h * (DH + 1):(h + 1) * (DH + 1)],
                    et_row[jt][:, ihh * 512:(ihh + 1) * 512],
                    start=(jt == av_jt_order[0]), stop=(jt == av_jt_order[-1]),
                )
                nmm -= 1
                if not st["mms"]:
                    pend_avs.pop(0)
                    queue_norm(h, bi, ot)

        norm_idx = [0]

        def emit_norm(h, bi, ot, half=None):
            if norm_fast:
                # Short chain: cross-partition-base DVE copy moves the psum
                # l-row (partition 64) straight to a partition-0 SBUF row (no
                # DMA hop), reciprocal there, broadcast, and the apply writes
                # odd-head AO rows at partition base 64 directly (no tmpo DMA).
                sl = slice(None) if half is None else slice(half * 512, (half + 1) * 512)
                lbf = lrp.tile([DH, n], f32, tag="lb", name="lbf") if half != 1 \
                    else emit_norm.lb_cur
                emit_norm.lb_cur = lbf
                lrow = lrp.tile([1, n], f32, tag="lrow", bufs=2, name="lrow") if half != 1 \
                    else emit_norm.lrow_cur
                emit_norm.lrow_cur = lrow
                nc.vector.tensor_copy(out=lrow[0:1, sl], in_=ot[DH:DH + 1, sl])
                if recip_scatter and half is None:
                    nsc = n // 128
                    lr0s = lrp.tile([128, nsc], f32, tag="lr0s", bufs=2, name="lr0s")
                    nc.sync.dma_start(out=lr0s[:], in_=lrow[0:1, :])
                    nc.vector.reciprocal_approx_fast(out=lr0s[:], in_=lr0s[:])
                    nc.sync.dma_start(out=lrow[0:1, :], in_=lr0s[:])
                else:
                    nc.vector.reciprocal_approx_fast(out=lrow[0:1, sl], in_=lrow[0:1, sl])
                nc.gpsimd.partition_broadcast(lbf[0:DH, sl], lrow[0:1, sl], channels=DH)
                po = (h % 2) * DH
                nc.vector.tensor_mul(
                    out=AO[bi][h // 2][po:po + DH, sl], in0=ot[0:DH, sl],
                    in1=lbf[0:DH, sl])
                return
            # HW-validated chain: copy the psum l-row (partition 64) to SBUF,
            # DMA it to partition 0, reciprocal there, broadcast from
            # partition 0. Custom DVE ops and partition_broadcast only work
            # from partition 0 on real hardware; ACT can read psum anywhere.
            sl = slice(None) if half is None else slice(half * 512, (half + 1) * 512)
            w = n if half is None else 512
            lb = lrp.tile([DH + 1, n], f32, tag="lb", name="lb") if half != 1 \
                else emit_norm.lb_cur
            emit_norm.lb_cur = lb
            if norm_idx[0] % 2 < row_copy_act:
                nc.scalar.activation(lb[DH:DH + 1, sl], ot[DH:DH + 1, sl], Copy)
            else:
                nc.vector.tensor_copy(out=lb[DH:DH + 1, sl], in_=ot[DH:DH + 1, sl])
            norm_idx[0] += 1
            if recip_scatter and half is None:
                # scatter the l row across 128 partitions, reciprocal there
                # (128x less DVE row time), gather back to a partition-0 row.
                nsc = n // 128
                lr0s = lrp.tile([128, nsc], f32, tag="lr0s", bufs=1, name="lr0s")
                nc.sync.dma_start(out=lr0s[:], in_=lb[DH:DH + 1, :])
                lrrs = lrp.tile([128, nsc], f32, tag="lrrs", bufs=1, name="lrrs")
                nc.vector.reciprocal_approx_fast(out=lrrs[:], in_=lr0s[:])
                lrr = lrp.tile([1, n], f32, tag="lrr", bufs=1, name="lrr")
                nc.sync.dma_start(out=lrr[0:1, :], in_=lrrs[:])
            else:
                lr0 = lrp.tile([1, n], f32, tag="lr0", bufs=1, name="lr0") if half != 1 \
                    else emit_norm.lr0_cur
                emit_norm.lr0_cur = lr0
                nc.sync.dma_start(out=lr0[0:1, sl], in_=lb[DH:DH + 1, sl])
                lrr = lrp.tile([1, n], f32, tag="lrr", bufs=1, name="lrr") if half != 1 \
                    else emit_norm.lrr_cur
                emit_norm.lrr_cur = lrr
                nc.vector.reciprocal_approx_fast(out=lrr[0:1, sl], in_=lr0[0:1, sl])
            nc.gpsimd.partition_broadcast(lb[0:DH, sl], lrr[0:1, sl], channels=DH)
            if h % 2 == 0:
                nc.vector.tensor_mul(
                    out=AO[bi][h // 2][0:DH, sl], in0=ot[0:DH, sl], in1=lb[0:DH, sl])
            else:
                tmpo = lrp.tile([DH, n], f16, tag="tmpo", bufs=tmpo_bufs, name="tmpo") \
                    if half != 1 else emit_norm.tmpo_cur
                emit_norm.tmpo_cur = tmpo
                nc.vector.tensor_mul(out=tmpo[0:DH, sl], in0=ot[0:DH, sl], in1=lb[0:DH, sl])
                nc.sync.dma_start(out=AO[bi][h // 2][DH:128, sl], in_=tmpo[0:DH, sl])

        # ---- prologue compute: QK projections for ip=0 (heads 0,1).
        # The sim ring is empty this early, so borrow it: four chunks then
        # rotate through four psum slots instead of two.
        for bi in range(bpc):
            emit_qk_chunk("q", QT, bi, 0, ring="mm" if proq_mm else None,
                          first_split=(bi == 0 and bool(head_split)))
            emit_qk_chunk("k", KT, bi, 0, ring="mm" if proq_mm else None,
                          first_split=(bi == 0 and bool(head_split)))

        # ---- deadline-scheduled inject: chunk -> emission slot
        sched_ip1 = list(sched_ip1)
        sched = {}
        slots_v0 = [0, 2, 4, 6]
        slots_v1 = [8, 10, 12, 14]
        slots_ip = {1: sched_ip1, 2: list(sched_ip2), 3: list(sched_ip3)}
        if split_inj:
            for t2 in range(T2):
                sched[slots_v0[t2]] = [("vh", 0, t2, 0), ("vh", 0, t2, 1)]
                sched[slots_v1[t2]] = [("vh", 1, t2, 0), ("vh", 1, t2, 1)]
            for ip in range(1, KP):
                for i, (kind, bi) in enumerate((("q", 0), ("k", 0), ("q", 1), ("k", 1))):
                    sched[slots_ip[ip][i]] = [(kind + "h", bi, ip, 0),
                                              (kind + "h", bi, ip, 1)]
        else:
            for t2 in range(T2):
                sched[slots_v0[t2]] = [("v", 0, t2)]
                sched[slots_v1[t2]] = [("v", 1, t2)]
            for ip in range(1, KP):
                for i, (kind, bi) in enumerate((("q", 0), ("k", 0), ("q", 1), ("k", 1))):
                    sched[slots_ip[ip][i]] = [(kind, bi, ip)]
        if f0_fine:
            for i, s in enumerate(sched_f0):
                sched.setdefault(s, []).append(("ff", 0, i))
        else:
            for i, t2 in enumerate(range(T2)):
                sched.setdefault(sched_f0[i], []).append(("f", 0, t2))

        def emit_chunk(c):
            kind = c[0]
            if kind == "v":
                emit_v_chunk(c[1], c[2])
            elif kind == "vh":
                emit_v_chunk(c[1], c[2], blocks=(c[3],))
            elif kind == "q":
                emit_qk_chunk("q", QT, c[1], c[2])
            elif kind == "k":
                emit_qk_chunk("k", KT, c[1], c[2])
            elif kind == "qh":
                emit_qk_chunk("q", QT, c[1], c[2], halves=(c[3],))
            elif kind == "kh":
                emit_qk_chunk("k", KT, c[1], c[2], halves=(c[3],))
            elif kind == "f":
                flush_norms()
                emit_f_chunk(c[1], c[2])
            elif kind == "ff":
                flush_norms()
                emit_f_nt(c[1], c[2])

        # ---- main attention loop; last head pair runs h-odd first so the
        # final stream is an even head (its norm-mul writes AO directly,
        # shortening the tail by one SBUF->SBUF DMA hop).
        stream_order = {}
        for hp in range(HP):
            h0, h1 = 2 * hp, 2 * hp + 1
            if hp == HP - 1:
                if hp3_order:
                    stream_order[hp] = [(h1, 0), (h0, 0), (h1, 1), (h0, 1)]
                else:
                    stream_order[hp] = [(h1, 0), (h1, 1), (h0, 0), (h0, 1)]
            else:
                stream_order[hp] = [(h0, 0), (h0, 1), (h1, 0), (h1, 1)]

        slot = 0
        for hp in range(HP):
            streams = stream_order[hp]
            first_h = streams[0][0]
            other_h = streams[2][0]
            for sidx, (h, bi) in enumerate(streams):
                is_last = (hp == HP - 1 and sidx == 3)
                ot_last = None
                if is_last:
                    ot_last = ps.tile([DH + 1, 1024], f32, tag="ot",
                                      bufs=ot_bufs, name="ot_last")
                et_row = []
                stream_state = {}
                if inline_av_lag and not is_last:
                    start_av(h, bi, et_row, inline=True)
                sp_hoist = [None]
                for jt in range(NT):
                    # eb prefetch (jt pairs): sidx1 fetches this pair's other
                    # head, sidx2 fetches the next head-pair's first head.
                    if jt % 2 == 0:
                        if hp3_order and hp == HP - 1:
                            if sidx == 0:
                                emit_eb_dma(streams[1][0], jt // 2)
                        elif sidx == 1:
                            emit_eb_dma(other_h, jt // 2)
                        if sidx == 2 and hp + 1 < HP:
                            if hp3_order and hp + 1 == HP - 1:
                                pass
                            emit_eb_dma(stream_order[hp + 1][0][0], jt // 2)
                    po = (h % 2) * 64

                    def emit_sim(j):
                        tag = "ot" if (sim7_ot and j >= NT - sim7_ot) else "mm"
                        spj = ps.tile([128, 1024], f32, tag=tag,
                                      bufs=(ot_bufs if tag == "ot" else sim_bufs),
                                      name="spj")
                        for ihh in range(2):
                            nc.tensor.matmul(
                                spj[:, ihh * 512:(ihh + 1) * 512],
                                KT[bi][hp][po:po + 64, j * 128:(j + 1) * 128],
                                QT[bi][hp][po:po + 64, ihh * 512:(ihh + 1) * 512],
                                start=True, stop=True,
                            )
                        return spj

                    if av_first and jt <= 1 and not inline_av_lag:
                        # ready av matmuls of the previous stream go ahead of
                        # the boundary sims (which wait on ACT freeing the
                        # sim psum ring) to avoid in-order head blocking.
                        step_av(av_first)
                    if jt == 1 and sp_hoist[0] is not None:
                        sp = sp_hoist[0]
                    else:
                        sp = emit_sim(jt)
                    if hoist_sim and jt == 0 and pend_any():
                        sp_hoist[0] = emit_sim(1)
                    if pair_mul:
                        if jt % 2 == 0:
                            eqp = etp.tile([128, 2, n], f16, tag="eq",
                                           bufs=max(2, eq_bufs // 2), name="eqp")
                            stream_state["eqp"] = eqp
                            nc.scalar.activation(eqp[:, 0, :], sp[:], Exp)
                        else:
                            eqp = stream_state["eqp"]
                            nc.scalar.activation(eqp[:, 1, :], sp[:], Exp)
                            etp2 = etp.tile([128, 2, n], f16, tag="et",
                                            bufs=max(2, et_bufs // 2), name="etp2")
                            nc.vector.tensor_mul(out=etp2[:], in0=eqp[:],
                                                 in1=eb_pairs[(h, jt // 2)])
                            et_row.append(etp2[:, 0, :])
                            et_row.append(etp2[:, 1, :])
                    else:
                        eq = etp.tile([128, n], f16, tag="eq", bufs=eq_bufs, name="eq")
                        nc.scalar.activation(eq[:], sp[:], Exp)
                        et = etp.tile([128, n], f16, tag="et", name="et")
                        use_pool = (not is_last) and jt in pool_jts
                        mul_eng = nc.gpsimd if use_pool else nc.vector
                        mul_eng.tensor_mul(out=et[:], in0=eq[:], in1=eb_tiles[(h, jt)])
                        et_row.append(et)
                    if is_last:
                        if pair_mul:
                            if jt % 2 == 1:
                                for j2 in (jt - 1, jt):
                                    for ihh in range(2):
                                        nc.tensor.matmul(
                                            ot_last[:, ihh * 512:(ihh + 1) * 512],
                                            VA[bi][j2][:, h * (DH + 1):(h + 1) * (DH + 1)],
                                            et_row[j2][:, ihh * 512:(ihh + 1) * 512],
                                            start=(j2 == 0), stop=(j2 == NT - 1),
                                        )
                        else:
                            for ihh in range(2):
                                nc.tensor.matmul(
                                    ot_last[:, ihh * 512:(ihh + 1) * 512],
                                    VA[bi][jt][:, h * (DH + 1):(h + 1) * (DH + 1)],
                                    et[:, ihh * 512:(ihh + 1) * 512],
                                    start=(jt == 0), stop=(jt == NT - 1),
                                )
                    flush_copies()
                    if jt == av_flush_slot and not inline_av_lag:
                        step_av(99)
                    if slot in sched:
                        for c in sched.pop(slot):
                            emit_chunk(c)
                    if inline_av_lag:
                        step_av(6)
                    if norm_delay and jt == norm_delay:
                        flush_norms()
                    slot += 1
                if not inline_av_lag:
                    step_av(99)  # flush any remaining AV of the previous stream
                if is_last:
                    if inline_av_lag:
                        step_av(99)  # drain all pending AV before the tail norm
                    flush_norms()
                    if last_norm_split:
                        emit_norm(h, bi, ot_last, half=0)
                        emit_norm(h, bi, ot_last, half=1)
                    else:
                        emit_norm(h, bi, ot_last)
                elif not inline_av_lag:
                    start_av(h, bi, et_row)
        step_av(99)
        flush_norms()
        # drain: anything left, then F for batch 1
        for s in sorted(sched):
            for c in sched[s]:
                emit_chunk(c)
        if drain_nt:
            for nt in range(NT):
                emit_f_nt(1, nt, ring=("mm" if (f_drain_mm and nt % 2) else None))
        else:
            for t2 in range(T2):
                emit_f_chunk(1, t2, ring=("mm" if (f_drain_mm and t2 % 2) else None),
                             fine=bool(drain_fine) and t2 == T2 - 1)

        for p in (ps, fop, lrp, etp, ebp, xtp, pers):
            p.release()

    nc.compile()
    return nc


def prep_inputs(x, Wq, Wk, Wv, rel_bias, Wo, bo, n=N, bpc=BPC, ncores=NCORES):
    """Host-side sharding/layout prep. Returns in_maps (one dict per core)."""
    f16 = np.float16
    x = np.asarray(x, dtype=np.float32)
    xT = np.ascontiguousarray(x.transpose(0, 2, 1)).astype(f16)   # [B, D, n]
    WqT = np.ascontiguousarray(Wq.T * np.float32(SCALE)).astype(f16)
    WkT = np.ascontiguousarray(Wk.T).astype(f16)
    WvT = np.ascontiguousarray(Wv.T).astype(f16)
    WoT = np.ascontiguousarray(Wo.T).astype(f16)
    expBT = np.ascontiguousarray(
        np.exp(np.asarray(rel_bias, dtype=np.float32).transpose(0, 2, 1))
    ).astype(f16)                                                  # [H, n(j), n(i)]
    in_maps = []
    for c in range(ncores):
        in_maps.append({
            "xT": np.ascontiguousarray(xT[c * bpc:(c + 1) * bpc]),
            "WqT": WqT, "WkT": WkT, "WvT": WvT, "WoT": WoT,
            "expBT": expBT,
        })
    return in_maps


BEST_KW = dict(norm_fast=1, f_drain_mm=1, norm_delay=6, hp3_order=1,
               f0_fine=1, sched_f0=(113, 114, 115, 117, 119, 121, 123, 125),
               eb_bufs=8, drain_nt=1, eq_bufs=10, et_bufs=16, last_norm_split=1,
               x0_split=1)
_CACHE = {}


def kernel(x, Wq, Wk, Wv, rel_bias, Wo, bo):
    from concourse.bass_utils import run_bass_kernel_spmd

    if "nc" not in _CACHE:
        _CACHE["nc"] = build_nc(**BEST_KW)
    nc = _CACHE["nc"]
    in_maps = prep_inputs(x, Wq, Wk, Wv, rel_bias, Wo, bo)
    res = run_bass_kernel_spmd(nc, in_maps, core_ids=list(range(NCORES)))
    out = np.concatenate([res.results[c]["out"] for c in range(NCORES)], axis=0)
    out = out + np.asarray(bo, dtype=np.float32)[None, None, :]
    return np.ascontiguousarray(out, dtype=np.float32)



# revision 33
# speedup vs baseline: 1.0524x; 1.0031x over previous
"""CrossAttention kernel for 8x TRN2 NeuronCores (Bass/Tile), v3.

Reference computation (per batch b of 16, heads h=8, n=1024, d_model=512, dh=64):
    q = x @ Wq.T, k = x @ Wk.T, v = x @ Wv.T          (per-head slices)
    sim = q k^T * scale + rel_bias[h]
    attn = softmax(sim, axis=-1)
    out = (attn @ v) re-assembled over heads, then @ Wo.T + bo

Sharding: data-parallel over batch, 2 batches per core x 8 cores.

v2 design notes (vs v1 baseline at ~264us modeled):
  - all matmul operands fp16 (full-rate on PE at any tile size, better
    mantissa than bf16, halves weight/x/eb DMA vs f32).
  - softmax runs on transposed sim (j on partitions); rel_bias applied as
    exp(sim)*exp(bias^T) with exp(bias^T) precomputed on host in fp16;
    the multiply runs in-place on DVE in 16-bit 2x mode.
  - attn@V uses V in natural layout as lhsT with an appended ones column:
    the same matmul emits the softmax denominator l as psum row 64.
  - normalization (HW constraint: custom DVE ops and partition_broadcast
    only operate from partition 0, and only on SBUF): DVE-copy the psum
    l-row to SBUF, DMA it to partition 0, reciprocal_approx_fast there,
    gpsimd partition_broadcast, one 1024-wide DVE multiply. Odd heads land
    in AO rows 64..127 via one SBUF->SBUF DMA.
  - output is fp16 (host converts to f32 and adds bias bo) - halves the
    serial output-DMA drain at the end of the program.
  - PSUM split into two rings: "mm" (sim tiles only, so ACT never starves
    behind injected work) and "ot" (attn@V accumulators + all projection /
    output-projection chunks).
  - coarse DMAs: one descriptor-chain per weight matrix / x batch, rel-bias
    loaded in jt-pairs - halves serial HWDGE occupancy.
  - emission is software-pipelined: projections for ip0 first, then the
    attention stream loop with V/QK(ip1..3)/F chunks injected between
    sim slots so the PE never starves while ACT grinds exp.

v3 changes (214.5us -> 203.8us modeled; PE busy floor is 168us):
  - norm_fast: the softmax-normalization chain is collapsed using
    cross-partition-base DVE ops (validated against this stack's
    executor): one DVE copy moves the psum l-row (partition 64) straight
    to a partition-0 SBUF row (the old copy + SBUF->SBUF DMA hop is
    gone), reciprocal_approx_fast runs in place, and the apply multiply
    writes odd heads' AO rows at partition base 64 directly (the old
    tmpo + DMA hop is gone).
  - norm_delay=6: norm-chain DVE ops of stream s are emitted 6 slots
    into stream s+1 so they do not sit in DVE's in-order queue ahead of
    the next stream's exp(sim)*exp(B) multiplies (which gate attn@V).
    The last head pair bypasses the delay (its AO feeds F promptly).
  - hp3_order: the last head pair runs (h7,b0),(h6,b0),(h7,b1),(h6,b1)
    so batch-0 AO completes two streams early, letting fine-grained
    (single-nt) F(0) chunks fill the otherwise chunk-starved stream
    14/15 boundaries (slots 113..125). eb prefetch adapted; eb ring 8.
  - last_norm_split halves the final stream's norm so F(1) starts on
    the first half; drain_nt emits the F(1) drain as 8 single-nt chunks
    alternating psum rings, shortening the serial tail.
  - f_drain_mm, x0_split, eq ring 10/et ring 16, ip1 chunk slots
    retuned (18,22,26,30).
"""

import numpy as np

HEADS = 8
DH = 64
B = 16
N = 1024
D = 512  # d_model = inner
SCALE = DH ** -0.5
NCORES = 8
BPC = B // NCORES  # batches per core


def build_nc(n=N, bpc=BPC, sim_bufs=2, ot_bufs=2, eb_bufs=7, et_bufs=18,
             lb_bufs=3, fo_bufs=6, tmpo_bufs=2, pool_mul_mod=0, row_copy_act=0, v_copy_act=0, last_norm_split=0, av_flush_slot=0, qk_copy_act_ip=-1, proq_mm=0, f_drain_mm=0, warmup_mms=10, x0_split=0, hoist_sim=1, recip_wide=0, split_inj=0, head_split=1, sched_ip1=(20, 24, 27, 30),
             f_copy_dve=0, recip_scatter=0, inline_av_lag=0, drain_fine=0,
             norm_fast=0, scr_dve=0, norm_delay=0,
             sched_ip2=(36, 44, 52, 60), sched_ip3=(68, 76, 84, 92),
             sched_f0=(120, 122, 124, 126), f0_fine=0, hp3_order=0,
             defer_qk_copy=0, pair_mul=0, drain_nt=0, eq_bufs=8, av_first=0,
             sim7_ot=0, split_tiles=0):
    import concourse.mybir as mybir
    import concourse.tile as tile
    from concourse import bacc

    f32 = mybir.dt.float32
    f16 = mybir.dt.float16
    Exp = mybir.ActivationFunctionType.Exp
    Copy = mybir.ActivationFunctionType.Copy

    NT = n // 128            # n tiles of 128 (8)
    KP = D // 128            # d_model k-tiles (4)
    HP = HEADS // 2          # head pairs (4)
    T2 = NT // 2             # double-nt chunks (4)

    nc = bacc.Bacc(None, target_bir_lowering=False)

    xT_d = nc.dram_tensor("xT", [bpc, D, n], f16, kind="ExternalInput")
    wq_d = nc.dram_tensor("WqT", [D, D], f16, kind="ExternalInput")   # pre-scaled
    wk_d = nc.dram_tensor("WkT", [D, D], f16, kind="ExternalInput")
    wv_d = nc.dram_tensor("WvT", [D, D], f16, kind="ExternalInput")
    wo_d = nc.dram_tensor("WoT", [D, D], f16, kind="ExternalInput")
    eb_d = nc.dram_tensor("expBT", [HEADS, n, n], f16, kind="ExternalInput")
    out_d = nc.dram_tensor("out", [bpc, n, D], f16, kind="ExternalOutput")

    with tile.TileContext(nc) as tc:
        pers = tc.alloc_tile_pool(name="pers", bufs=1)
        # ---- persistent tiles
        QT = [[pers.tile([128, n], f16, tag=f"qt{bi}_{ip}", name=f"qt{bi}_{ip}")
               for ip in range(KP)] for bi in range(bpc)]
        KT = [[pers.tile([128, n], f16, tag=f"kt{bi}_{ip}", name=f"kt{bi}_{ip}")
               for ip in range(KP)] for bi in range(bpc)]
        VA = [[pers.tile([128, HEADS * (DH + 1)], f16, tag=f"va{bi}_{nt}",
                         name=f"va{bi}_{nt}") for nt in range(NT)]
              for bi in range(bpc)]
        AO = [[pers.tile([128, n], f16, tag=f"ao{bi}_{kp}", name=f"ao{bi}_{kp}")
               for kp in range(KP)] for bi in range(bpc)]
        # each weight matrix lives in one [128, KP, 512] tile (one DMA each)
        # (or per-kp tiles when split_tiles, so first-chunk matmuls can start
        # as soon as their kp slice of W and x has landed)
        w4 = {}
        for wname, wd in (("q", wq_d), ("k", wk_d), ("v", wv_d), ("o", wo_d)):
            if split_tiles:
                w4[wname] = [pers.tile([128, D], f16, tag=f"w{wname}{kp}",
                                       name=f"w{wname}{kp}") for kp in range(KP)]
            else:
                w4[wname] = pers.tile([128, KP, D], f16, tag=f"w{wname}",
                                      name=f"w{wname}")
        xtp = tc.alloc_tile_pool(name="xt", bufs=1)
        if split_tiles:
            xt = [[xtp.tile([128, n], f16, tag=f"x{bi}_{kp}", name=f"x{bi}_{kp}")
                   for kp in range(KP)] for bi in range(bpc)]
        else:
            xt = [xtp.tile([128, KP, n], f16, tag=f"x{bi}", name=f"x{bi}")
                  for bi in range(bpc)]

        def w_ap(wname, kp, sl=slice(None)):
            if split_tiles:
                return w4[wname][kp][:, sl]
            return w4[wname][:, kp, sl]

        def x_ap(bi, kp, sl=slice(None)):
            if split_tiles:
                return xt[bi][kp][:, sl]
            return xt[bi][:, kp, sl]

        ebp = tc.alloc_tile_pool(name="eb", bufs=eb_bufs)
        etp = tc.alloc_tile_pool(name="et", bufs=et_bufs)
        lrp = tc.alloc_tile_pool(name="lr", bufs=lb_bufs)
        fop = tc.alloc_tile_pool(name="fop", bufs=1)
        ps = tc.alloc_tile_pool(name="ps", bufs=1, space="PSUM")

        # ---- prologue DMAs (one chained DMA per tensor)
        def dma_w(wname, wd, kp=None):
            if split_tiles:
                for k in ([kp] if kp is not None else range(KP)):
                    nc.sync.dma_start(out=w4[wname][k][:],
                                      in_=wd[k * 128:(k + 1) * 128, :])
                return
            dst = w4[wname][:]
            src = wd.rearrange("(kp p) c -> p kp c", p=128)
            nc.sync.dma_start(out=dst, in_=src)

        def dma_x(bi, split=False, kp=None):
            if split_tiles:
                for k in ([kp] if kp is not None else range(KP)):
                    nc.sync.dma_start(out=xt[bi][k][:],
                                      in_=xT_d[bi, k * 128:(k + 1) * 128, :])
                return
            if split:
                for kp_ in range(KP):
                    nc.sync.dma_start(
                        out=xt[bi][:, kp_, :],
                        in_=xT_d[bi, kp_ * 128:(kp_ + 1) * 128, :])
            else:
                nc.sync.dma_start(
                    out=xt[bi][:], in_=xT_d[bi].rearrange("(kp p) j -> p kp j", p=128))

        pool_jts = set(range(3, 3 + pool_mul_mod))
        av_jt_order = [j for j in range(NT) if j not in pool_jts] + sorted(pool_jts)
        eb_tiles = {}
        eb_pairs = {}

        def emit_eb_dma(h, jp):
            """Load jt pair (2*jp, 2*jp+1) of head h as one [128, 2, n] tile."""
            t = ebp.tile([128, 2, n], f16, tag="eb", name="eb")
            nc.sync.dma_start(
                out=t[:],
                in_=eb_d[h, 2 * jp * 128:(2 * jp + 2) * 128, :].rearrange(
                    "(two p) i -> p two i", two=2))
            eb_tiles[(h, 2 * jp)] = t[:, 0, :]
            eb_tiles[(h, 2 * jp + 1)] = t[:, 1, :]
            eb_pairs[(h, jp)] = t[:]

        if split_tiles:
            for kp in range(KP):
                dma_w("q", wq_d, kp=kp)
                dma_x(0, kp=kp)
            dma_w("k", wk_d)
            dma_x(1)
            dma_w("v", wv_d)
            dma_w("o", wo_d)
        else:
            dma_w("q", wq_d)
            dma_x(0, split=bool(x0_split))
            dma_w("k", wk_d)
            dma_x(1)
            dma_w("v", wv_d)
            dma_w("o", wo_d)
        for jp in range(NT // 2):
            emit_eb_dma(0, jp)

        # ---- PE warm-up: keep the tensor engine continuously busy through
        # the prologue DMAs so its p-state clock is fully ramped (213ns/row
        # instead of 427+) when the first real projection matmuls arrive.
        if warmup_mms:
            scr = pers.tile([128, 512], f16, tag="scr", name="scr")
            if scr_dve:
                nc.vector.memset(scr[:], 0.0)
            else:
                nc.gpsimd.memset(scr[:], 0.0)
            wps = ps.tile([128, 512], f32, tag="mm", bufs=sim_bufs, name="wps")
            for _ in range(warmup_mms):
                nc.tensor.matmul(wps[:], scr[:, 0:128], scr[:],
                                 start=True, stop=True)

        # ---- emitters -----------------------------------------------------
        inj_ring = [0]
        pend_copies = []

        def flush_copies():
            while pend_copies:
                pend_copies.pop(0)()

        def inj_tag():
            return "ot"

        def emit_qk_chunk(wname, DST, bi, ip, ring=None, halves=(0, 1), first_split=False):
            W_s = w4[wname]
            wide = len(halves) == 2 and not split_inj
            pt = ps.tile([128, 1024 if wide else 512], f32,
                         tag=ring or inj_tag(), bufs=ot_bufs, name="pt")
            for i, nh in enumerate(halves):
                base = i * 512 if wide else 0
                for kp in range(KP):
                    nc.tensor.matmul(
                        pt[:, base:base + 512],
                        w_ap(wname, kp, slice(ip * 128, (ip + 1) * 128)),
                        x_ap(bi, kp, slice(nh * 512, (nh + 1) * 512)),
                        start=(kp == 0), stop=(kp == KP - 1),
                    )
                if not wide:
                    nc.vector.tensor_copy(
                        out=DST[bi][ip][:, nh * 512:(nh + 1) * 512], in_=pt[:])
            if wide:
                if first_split:
                    # sim jt0 needs only K cols 0:128 / Q cols 0:512 - copy
                    # those first so the first sim fires sooner.
                    c0 = 128 if wname == "k" else 512
                    nc.vector.tensor_copy(out=DST[bi][ip][:, 0:c0], in_=pt[:, 0:c0])
                    nc.vector.tensor_copy(out=DST[bi][ip][:, c0:], in_=pt[:, c0:])
                elif defer_qk_copy and ip > 0:
                    pend_copies.append(
                        lambda d=DST[bi][ip], s=pt: nc.vector.tensor_copy(out=d[:], in_=s[:]))
                else:
                    nc.vector.tensor_copy(out=DST[bi][ip][:], in_=pt[:])

        def emit_v_chunk(bi, t2, blocks=(0, 1)):
            wide = len(blocks) == 2 and not split_inj
            pt = ps.tile([128, 1024 if wide else 512], f32, tag=inj_tag(),
                         bufs=ot_bufs, name="pt")
            for i, b in enumerate(blocks):
                nt = 2 * t2 + b
                base = i * 512 if wide else 0
                for kp in range(KP):
                    nc.tensor.matmul(
                        pt[:, base:base + 512],
                        x_ap(bi, kp, slice(nt * 128, (nt + 1) * 128)),
                        w_ap("v", kp),
                        start=(kp == 0), stop=(kp == KP - 1),
                    )
                va = VA[bi][nt]
                nc.gpsimd.memset(va[:], 1.0)
                dst3 = va[:].rearrange("p (h c) -> p h c", c=DH + 1)[:, :, 0:DH]
                src3 = pt[:, base:base + 512].rearrange("p (h c) -> p h c", c=DH)
                nc.vector.tensor_copy(out=dst3, in_=src3)

        def emit_f_drain(pt, bi, nt, b):
            fo = fop.tile([128, 512], f16, tag="fo", bufs=fo_bufs, name="fo")
            if f_copy_dve:
                nc.vector.tensor_copy(out=fo[:], in_=pt[:, b * 512:(b + 1) * 512])
            else:
                nc.scalar.activation(fo[:], pt[:, b * 512:(b + 1) * 512], Copy)
            nc.sync.dma_start(out=out_d[bi, nt * 128:(nt + 1) * 128, :], in_=fo[:])

        def emit_f_chunk(bi, t2, ring=None, fine=False):
            pt = ps.tile([128, 1024], f32, tag=ring or inj_tag(), bufs=ot_bufs,
                         name="pt")
            for b in range(2):
                nt = 2 * t2 + b
                for kp in range(KP):
                    nc.tensor.matmul(
                        pt[:, b * 512:(b + 1) * 512],
                        AO[bi][kp][:, nt * 128:(nt + 1) * 128],
                        w_ap("o", kp),
                        start=(kp == 0), stop=(kp == KP - 1),
                    )
                if fine:
                    emit_f_drain(pt, bi, nt, b)
            if not fine:
                for b in range(2):
                    emit_f_drain(pt, bi, 2 * t2 + b, b)

        def emit_f_nt(bi, nt, ring=None):
            pt = ps.tile([128, 512], f32, tag=ring or inj_tag(), bufs=ot_bufs, name="ptf")
            for kp in range(KP):
                nc.tensor.matmul(
                    pt[:, 0:512],
                    AO[bi][kp][:, nt * 128:(nt + 1) * 128],
                    w_ap("o", kp),
                    start=(kp == 0), stop=(kp == KP - 1),
                )
            emit_f_drain(pt, bi, nt, 0)

        pend_avs = []
        pend_norms = []

        def pend_any():
            return bool(pend_avs)

        def queue_norm(h, bi, ot):
            if norm_delay and (h // 2) != HP - 1:
                pend_norms.append((h, bi, ot))
            else:
                flush_norms()
                emit_norm(h, bi, ot)

        def flush_norms():
            while pend_norms:
                emit_norm(*pend_norms.pop(0))

        def start_av(h, bi, et_row, inline=False):
            ot = ps.tile([DH + 1, 1024], f32, tag="ot", bufs=ot_bufs, name="ot")
            if inline:
                mms = [(ihh, jt) for jt in av_jt_order for ihh in range(2)]
            else:
                mms = [(ihh, jt) for ihh in range(2) for jt in av_jt_order]
            pend_avs.append({"h": h, "bi": bi, "et": et_row, "ot": ot,
                             "mms": mms, "lag": inline_av_lag if inline else 0})

        def step_av(nmm):
            while nmm > 0 and pend_avs:
                st = pend_avs[0]
                h, bi, et_row, ot = st["h"], st["bi"], st["et"], st["ot"]
                if not st["mms"]:
                    pend_avs.pop(0)
                    queue_norm(h, bi, ot)
                    continue
                ihh, jt = st["mms"][0]
                lag = st["lag"] if st is pend_avs[-1] else 0
                if jt >= len(et_row) - lag:
                    return
                st["mms"].pop(0)
                nc.tensor.matmul(
                    ot[:, ihh * 512:(ihh + 1) * 512],
                    VA[bi][jt][:, h * (DH + 1):(h + 1) * (DH + 1)],
                    et_row[jt][:, ihh * 512:(ihh + 1) * 512],
                    start=(jt == av_jt_order[0]), stop=(jt == av_jt_order[-1]),
                )
                nmm -= 1
                if not st["mms"]:
                    pend_avs.pop(0)
                    queue_norm(h, bi, ot)

        norm_idx = [0]

        def emit_norm(h, bi, ot, half=None):
            if norm_fast:
                # Short chain: cross-partition-base DVE copy moves the psum
                # l-row (partition 64) straight to a partition-0 SBUF row (no
                # DMA hop), reciprocal there, broadcast, and the apply writes
                # odd-head AO rows at partition base 64 directly (no tmpo DMA).
                sl = slice(None) if half is None else slice(half * 512, (half + 1) * 512)
                lbf = lrp.tile([DH, n], f32, tag="lb", name="lbf") if half != 1 \
                    else emit_norm.lb_cur
                emit_norm.lb_cur = lbf
                lrow = lrp.tile([1, n], f32, tag="lrow", bufs=2, name="lrow") if half != 1 \
                    else emit_norm.lrow_cur
                emit_norm.lrow_cur = lrow
                nc.vector.tensor_copy(out=lrow[0:1, sl], in_=ot[DH:DH + 1, sl])
                if recip_scatter and half is None:
                    nsc = n // 128
                    lr0s = lrp.tile([128, nsc], f32, tag="lr0s", bufs=2, name="lr0s")
                    nc.sync.dma_start(out=lr0s[:], in_=lrow[0:1, :])
                    nc.vector.reciprocal_approx_fast(out=lr0s[:], in_=lr0s[:])
                    nc.sync.dma_start(out=lrow[0:1, :], in_=lr0s[:])
                else:
                    nc.vector.reciprocal_approx_fast(out=lrow[0:1, sl], in_=lrow[0:1, sl])
                nc.gpsimd.partition_broadcast(lbf[0:DH, sl], lrow[0:1, sl], channels=DH)
                po = (h % 2) * DH
                nc.vector.tensor_mul(
                    out=AO[bi][h // 2][po:po + DH, sl], in0=ot[0:DH, sl],
                    in1=lbf[0:DH, sl])
                return
            # HW-validated chain: copy the psum l-row (partition 64) to SBUF,
            # DMA it to partition 0, reciprocal there, broadcast from
            # partition 0. Custom DVE ops and partition_broadcast only work
            # from partition 0 on real hardware; ACT can read psum anywhere.
            sl = slice(None) if half is None else slice(half * 512, (half + 1) * 512)
            w = n if half is None else 512
            lb = lrp.tile([DH + 1, n], f32, tag="lb", name="lb") if half != 1 \
                else emit_norm.lb_cur
            emit_norm.lb_cur = lb
            if norm_idx[0] % 2 < row_copy_act:
                nc.scalar.activation(lb[DH:DH + 1, sl], ot[DH:DH + 1, sl], Copy)
            else:
                nc.vector.tensor_copy(out=lb[DH:DH + 1, sl], in_=ot[DH:DH + 1, sl])
            norm_idx[0] += 1
            if recip_scatter and half is None:
                # scatter the l row across 128 partitions, reciprocal there
                # (128x less DVE row time), gather back to a partition-0 row.
                nsc = n // 128
                lr0s = lrp.tile([128, nsc], f32, tag="lr0s", bufs=1, name="lr0s")
                nc.sync.dma_start(out=lr0s[:], in_=lb[DH:DH + 1, :])
                lrrs = lrp.tile([128, nsc], f32, tag="lrrs", bufs=1, name="lrrs")
                nc.vector.reciprocal_approx_fast(out=lrrs[:], in_=lr0s[:])
                lrr = lrp.tile([1, n], f32, tag="lrr", bufs=1, name="lrr")
                nc.sync.dma_start(out=lrr[0:1, :], in_=lrrs[:])
            else:
                lr0 = lrp.tile([1, n], f32, tag="lr0", bufs=1, name="lr0") if half != 1 \
                    else emit_norm.lr0_cur
                emit_norm.lr0_cur = lr0
                nc.sync.dma_start(out=lr0[0:1, sl], in_=lb[DH:DH + 1, sl])
                lrr = lrp.tile([1, n], f32, tag="lrr", bufs=1, name="lrr") if half != 1 \
                    else emit_norm.lrr_cur
                emit_norm.lrr_cur = lrr
                nc.vector.reciprocal_approx_fast(out=lrr[0:1, sl], in_=lr0[0:1, sl])
            nc.gpsimd.partition_broadcast(lb[0:DH, sl], lrr[0:1, sl], channels=DH)
            if h % 2 == 0:
                nc.vector.tensor_mul(
                    out=AO[bi][h // 2][0:DH, sl], in0=ot[0:DH, sl], in1=lb[0:DH, sl])
            else:
                tmpo = lrp.tile([DH, n], f16, tag="tmpo", bufs=tmpo_bufs, name="tmpo") \
                    if half != 1 else emit_norm.tmpo_cur
                emit_norm.tmpo_cur = tmpo
                nc.vector.tensor_mul(out=tmpo[0:DH, sl], in0=ot[0:DH, sl], in1=lb[0:DH, sl])
                nc.sync.dma_start(out=AO[bi][h // 2][DH:128, sl], in_=tmpo[0:DH, sl])

        # ---- prologue compute: QK projections for ip=0 (heads 0,1).
        # The sim ring is empty this early, so borrow it: four chunks then
        # rotate through four psum slots instead of two.
        for bi in range(bpc):
            emit_qk_chunk("q", QT, bi, 0, ring="mm" if proq_mm else None,
                          first_split=(bi == 0 and bool(head_split)))
            emit_qk_chunk("k", KT, bi, 0, ring="mm" if proq_mm else None,
                          first_split=(bi == 0 and bool(head_split)))

        # ---- deadline-scheduled inject: chunk -> emission slot
        sched_ip1 = list(sched_ip1)
        sched = {}
        slots_v0 = [0, 2, 4, 6]
        slots_v1 = [8, 10, 12, 14]
        slots_ip = {1: sched_ip1, 2: list(sched_ip2), 3: list(sched_ip3)}
        if split_inj:
            for t2 in range(T2):
                sched[slots_v0[t2]] = [("vh", 0, t2, 0), ("vh", 0, t2, 1)]
                sched[slots_v1[t2]] = [("vh", 1, t2, 0), ("vh", 1, t2, 1)]
            for ip in range(1, KP):
                for i, (kind, bi) in enumerate((("q", 0), ("k", 0), ("q", 1), ("k", 1))):
                    sched[slots_ip[ip][i]] = [(kind + "h", bi, ip, 0),
                                              (kind + "h", bi, ip, 1)]
        else:
            for t2 in range(T2):
                sched[slots_v0[t2]] = [("v", 0, t2)]
                sched[slots_v1[t2]] = [("v", 1, t2)]
            for ip in range(1, KP):
                for i, (kind, bi) in enumerate((("q", 0), ("k", 0), ("q", 1), ("k", 1))):
                    sched[slots_ip[ip][i]] = [(kind, bi, ip)]
        if f0_fine:
            for i, s in enumerate(sched_f0):
                sched.setdefault(s, []).append(("ff", 0, i))
        else:
            for i, t2 in enumerate(range(T2)):
                sched.setdefault(sched_f0[i], []).append(("f", 0, t2))

        def emit_chunk(c):
            kind = c[0]
            if kind == "v":
                emit_v_chunk(c[1], c[2])
            elif kind == "vh":
                emit_v_chunk(c[1], c[2], blocks=(c[3],))
            elif kind == "q":
                emit_qk_chunk("q", QT, c[1], c[2])
            elif kind == "k":
                emit_qk_chunk("k", KT, c[1], c[2])
            elif kind == "qh":
                emit_qk_chunk("q", QT, c[1], c[2], halves=(c[3],))
            elif kind == "kh":
                emit_qk_chunk("k", KT, c[1], c[2], halves=(c[3],))
            elif kind == "f":
                flush_norms()
                emit_f_chunk(c[1], c[2])
            elif kind == "ff":
                flush_norms()
                emit_f_nt(c[1], c[2])

        # ---- main attention loop; last head pair runs h-odd first so the
        # final stream is an even head (its norm-mul writes AO directly,
        # shortening the tail by one SBUF->SBUF DMA hop).
        stream_order = {}
        for hp in range(HP):
            h0, h1 = 2 * hp, 2 * hp + 1
            if hp == HP - 1:
                if hp3_order:
                    stream_order[hp] = [(h1, 0), (h0, 0), (h1, 1), (h0, 1)]
                else:
                    stream_order[hp] = [(h1, 0), (h1, 1), (h0, 0), (h0, 1)]
            else:
                stream_order[hp] = [(h0, 0), (h0, 1), (h1, 0), (h1, 1)]

        slot = 0
        for hp in range(HP):
            streams = stream_order[hp]
            first_h = streams[0][0]
            other_h = streams[2][0]
            for sidx, (h, bi) in enumerate(streams):
                is_last = (hp == HP - 1 and sidx == 3)
                ot_last = None
                if is_last:
                    ot_last = ps.tile([DH + 1, 1024], f32, tag="ot",
                                      bufs=ot_bufs, name="ot_last")
                et_row = []
                stream_state = {}
                if inline_av_lag and not is_last:
                    start_av(h, bi, et_row, inline=True)
                sp_hoist = [None]
                for jt in range(NT):
                    # eb prefetch (jt pairs): sidx1 fetches this pair's other
                    # head, sidx2 fetches the next head-pair's first head.
                    if jt % 2 == 0:
                        if hp3_order and hp == HP - 1:
                            if sidx == 0:
                                emit_eb_dma(streams[1][0], jt // 2)
                        elif sidx == 1:
                            emit_eb_dma(other_h, jt // 2)
                        if sidx == 2 and hp + 1 < HP:
                            if hp3_order and hp + 1 == HP - 1:
                                pass
                            emit_eb_dma(stream_order[hp + 1][0][0], jt // 2)
                    po = (h % 2) * 64

                    def emit_sim(j):
                        tag = "ot" if (sim7_ot and j >= NT - sim7_ot) else "mm"
                        spj = ps.tile([128, 1024], f32, tag=tag,
                                      bufs=(ot_bufs if tag == "ot" else sim_bufs),
                                      name="spj")
                        for ihh in range(2):
                            nc.tensor.matmul(
                                spj[:, ihh * 512:(ihh + 1) * 512],
                                KT[bi][hp][po:po + 64, j * 128:(j + 1) * 128],
                                QT[bi][hp][po:po + 64, ihh * 512:(ihh + 1) * 512],
                                start=True, stop=True,
                            )
                        return spj

                    if av_first and jt <= 1 and not inline_av_lag:
                        # ready av matmuls of the previous stream go ahead of
                        # the boundary sims (which wait on ACT freeing the
                        # sim psum ring) to avoid in-order head blocking.
                        step_av(av_first)
                    if jt == 1 and sp_hoist[0] is not None:
                        sp = sp_hoist[0]
                    else:
                        sp = emit_sim(jt)
                    if hoist_sim and jt == 0 and pend_any():
                        sp_hoist[0] = emit_sim(1)
                    if pair_mul:
                        if jt % 2 == 0:
                            eqp = etp.tile([128, 2, n], f16, tag="eq",
                                           bufs=max(2, eq_bufs // 2), name="eqp")
                            stream_state["eqp"] = eqp
                            nc.scalar.activation(eqp[:, 0, :], sp[:], Exp)
                        else:
                            eqp = stream_state["eqp"]
                            nc.scalar.activation(eqp[:, 1, :], sp[:], Exp)
                            etp2 = etp.tile([128, 2, n], f16, tag="et",
                                            bufs=max(2, et_bufs // 2), name="etp2")
                            nc.vector.tensor_mul(out=etp2[:], in0=eqp[:],
                                                 in1=eb_pairs[(h, jt // 2)])
                            et_row.append(etp2[:, 0, :])
                            et_row.append(etp2[:, 1, :])
                    else:
                        eq = etp.tile([128, n], f16, tag="eq", bufs=eq_bufs, name="eq")
                        nc.scalar.activation(eq[:], sp[:], Exp)
                        et = etp.tile([128, n], f16, tag="et", name="et")
                        use_pool = (not is_last) and jt in pool_jts
                        mul_eng = nc.gpsimd if use_pool else nc.vector
                        mul_eng.tensor_mul(out=et[:], in0=eq[:], in1=eb_tiles[(h, jt)])
                        et_row.append(et)
                    if is_last:
                        if pair_mul:
                            if jt % 2 == 1:
                                for j2 in (jt - 1, jt):
                                    for ihh in range(2):
                                        nc.tensor.matmul(
                                            ot_last[:, ihh * 512:(ihh + 1) * 512],
                                            VA[bi][j2][:, h * (DH + 1):(h + 1) * (DH + 1)],
                                            et_row[j2][:, ihh * 512:(ihh + 1) * 512],
                                            start=(j2 == 0), stop=(j2 == NT - 1),
                                        )
                        else:
                            for ihh in range(2):
                                nc.tensor.matmul(
                                    ot_last[:, ihh * 512:(ihh + 1) * 512],
                                    VA[bi][jt][:, h * (DH + 1):(h + 1) * (DH + 1)],
                                    et[:, ihh * 512:(ihh + 1) * 512],
                                    start=(jt == 0), stop=(jt == NT - 1),
                                )
                    flush_copies()
                    if jt == av_flush_slot and not inline_av_lag:
                        step_av(99)
                    if slot in sched:
                        for c in sched.pop(slot):
                            emit_chunk(c)
                    if inline_av_lag:
                        step_av(6)
                    if norm_delay and jt == norm_delay:
                        flush_norms()
                    slot += 1
                if not inline_av_lag:
                    step_av(99)  # flush any remaining AV of the previous stream
                if is_last:
                    if inline_av_lag:
                        step_av(99)  # drain all pending AV before the tail norm
                    flush_norms()
                    if last_norm_split:
                        emit_norm(h, bi, ot_last, half=0)
                        emit_norm(h, bi, ot_last, half=1)
                    else:
                        emit_norm(h, bi, ot_last)
                elif not inline_av_lag:
                    start_av(h, bi, et_row)
        step_av(99)
        flush_norms()
        # drain: anything left, then F for batch 1
        for s in sorted(sched):
            for c in sched[s]:
                emit_chunk(c)
        if drain_nt:
            for nt in range(NT):
                emit_f_nt(1, nt, ring=("mm" if (f_drain_mm and nt % 2) else None))
        else:
            for t2 in range(T2):
                emit_f_chunk(1, t2, ring=("mm" if (f_drain_mm and t2 % 2) else None),
                             fine=bool(drain_fine) and t2 == T2 - 1)

        for p in (ps, fop, lrp, etp, ebp, xtp, pers):
            p.release()

    nc.compile()
    return nc


def prep_inputs(x, Wq, Wk, Wv, rel_bias, Wo, bo, n=N, bpc=BPC, ncores=NCORES):
    """Host-side sharding/layout prep. Returns in_maps (one dict per core)."""
    f16 = np.float16
    x = np.asarray(x, dtype=np.float32)
    xT = np.ascontiguousarray(x.transpose(0, 2, 1)).astype(f16)   # [B, D, n]
    WqT = np.ascontiguousarray(Wq.T * np.float32(SCALE)).astype(f16)
    WkT = np.ascontiguousarray(Wk.T).astype(f16)
    WvT = np.ascontiguousarray(Wv.T).astype(f16)
    WoT = np.ascontiguousarray(Wo.T).astype(f16)
    expBT = np.ascontiguousarray(
        np.exp(np.asarray(rel_bias, dtype=np.float32).transpose(0, 2, 1))
    ).astype(f16)                                                  # [H, n(j), n(i)]
    in_maps = []
    for c in range(ncores):
        in_maps.append({
            "xT": np.ascontiguousarray(xT[c * bpc:(c + 1) * bpc]),
            "WqT": WqT, "WkT": WkT, "WvT": WvT, "WoT": WoT,
            "expBT": expBT,
        })
    return in_maps


BEST_KW = dict(norm_fast=1, f_drain_mm=1, norm_delay=6, hp3_order=1,
               f0_fine=1, sched_f0=(113, 114, 115, 117, 119, 121, 123, 125),
               eb_bufs=8, drain_nt=1, eq_bufs=10, et_bufs=16, last_norm_split=1,
               x0_split=1, sched_ip1=(18, 22, 26, 30))
_CACHE = {}


def kernel(x, Wq, Wk, Wv, rel_bias, Wo, bo):
    from concourse.bass_utils import run_bass_kernel_spmd

    if "nc" not in _CACHE:
        _CACHE["nc"] = build_nc(**BEST_KW)
    nc = _CACHE["nc"]
    in_maps = prep_inputs(x, Wq, Wk, Wv, rel_bias, Wo, bo)
    res = run_bass_kernel_spmd(nc, in_maps, core_ids=list(range(NCORES)))
    out = np.concatenate([res.results[c]["out"] for c in range(NCORES)], axis=0)
    out = out + np.asarray(bo, dtype=np.float32)[None, None, :]
    return np.ascontiguousarray(out, dtype=np.float32)



# revision 36
# speedup vs baseline: 1.0535x; 1.0011x over previous
"""CrossAttention kernel for 8x TRN2 NeuronCores (Bass/Tile), v3.

Reference computation (per batch b of 16, heads h=8, n=1024, d_model=512, dh=64):
    q = x @ Wq.T, k = x @ Wk.T, v = x @ Wv.T          (per-head slices)
    sim = q k^T * scale + rel_bias[h]
    attn = softmax(sim, axis=-1)
    out = (attn @ v) re-assembled over heads, then @ Wo.T + bo

Sharding: data-parallel over batch, 2 batches per core x 8 cores.

v2 design notes (vs v1 baseline at ~264us modeled):
  - all matmul operands fp16 (full-rate on PE at any tile size, better
    mantissa than bf16, halves weight/x/eb DMA vs f32).
  - softmax runs on transposed sim (j on partitions); rel_bias applied as
    exp(sim)*exp(bias^T) with exp(bias^T) precomputed on host in fp16;
    the multiply runs in-place on DVE in 16-bit 2x mode.
  - attn@V uses V in natural layout as lhsT with an appended ones column:
    the same matmul emits the softmax denominator l as psum row 64.
  - normalization (HW constraint: custom DVE ops and partition_broadcast
    only operate from partition 0, and only on SBUF): DVE-copy the psum
    l-row to SBUF, DMA it to partition 0, reciprocal_approx_fast there,
    gpsimd partition_broadcast, one 1024-wide DVE multiply. Odd heads land
    in AO rows 64..127 via one SBUF->SBUF DMA.
  - output is fp16 (host converts to f32 and adds bias bo) - halves the
    serial output-DMA drain at the end of the program.
  - PSUM split into two rings: "mm" (sim tiles only, so ACT never starves
    behind injected work) and "ot" (attn@V accumulators + all projection /
    output-projection chunks).
  - coarse DMAs: one descriptor-chain per weight matrix / x batch, rel-bias
    loaded in jt-pairs - halves serial HWDGE occupancy.
  - emission is software-pipelined: projections for ip0 first, then the
    attention stream loop with V/QK(ip1..3)/F chunks injected between
    sim slots so the PE never starves while ACT grinds exp.

v3 changes (214.5us -> 203.8us modeled; PE busy floor is 168us):
  - norm_fast: the softmax-normalization chain is collapsed using
    cross-partition-base DVE ops (validated against this stack's
    executor): one DVE copy moves the psum l-row (partition 64) straight
    to a partition-0 SBUF row (the old copy + SBUF->SBUF DMA hop is
    gone), reciprocal_approx_fast runs in place, and the apply multiply
    writes odd heads' AO rows at partition base 64 directly (the old
    tmpo + DMA hop is gone).
  - norm_delay=6: norm-chain DVE ops of stream s are emitted 6 slots
    into stream s+1 so they do not sit in DVE's in-order queue ahead of
    the next stream's exp(sim)*exp(B) multiplies (which gate attn@V).
    The last head pair bypasses the delay (its AO feeds F promptly).
  - hp3_order: the last head pair runs (h7,b0),(h6,b0),(h7,b1),(h6,b1)
    so batch-0 AO completes two streams early, letting fine-grained
    (single-nt) F(0) chunks fill the otherwise chunk-starved stream
    14/15 boundaries (slots 113..125). eb prefetch adapted; eb ring 8.
  - last_norm_split halves the final stream's norm so F(1) starts on
    the first half; drain_nt emits the F(1) drain as 8 single-nt chunks
    alternating psum rings, shortening the serial tail.
  - f_drain_mm, x0_split, eq ring 10/et ring 16, ip1 chunk slots
    retuned (18,22,26,30); av_first=1 lets one ready attn@V matmul of
    the previous stream jump ahead of the boundary sims in the PE queue.
"""

import numpy as np

HEADS = 8
DH = 64
B = 16
N = 1024
D = 512  # d_model = inner
SCALE = DH ** -0.5
NCORES = 8
BPC = B // NCORES  # batches per core


def build_nc(n=N, bpc=BPC, sim_bufs=2, ot_bufs=2, eb_bufs=7, et_bufs=18,
             lb_bufs=3, fo_bufs=6, tmpo_bufs=2, pool_mul_mod=0, row_copy_act=0, v_copy_act=0, last_norm_split=0, av_flush_slot=0, qk_copy_act_ip=-1, proq_mm=0, f_drain_mm=0, warmup_mms=10, x0_split=0, hoist_sim=1, recip_wide=0, split_inj=0, head_split=1, sched_ip1=(20, 24, 27, 30),
             f_copy_dve=0, recip_scatter=0, inline_av_lag=0, drain_fine=0,
             norm_fast=0, scr_dve=0, norm_delay=0,
             sched_ip2=(36, 44, 52, 60), sched_ip3=(68, 76, 84, 92),
             sched_f0=(120, 122, 124, 126), f0_fine=0, hp3_order=0,
             defer_qk_copy=0, pair_mul=0, drain_nt=0, eq_bufs=8, av_first=0,
             sim7_ot=0, split_tiles=0, drain_pre=0):
    import concourse.mybir as mybir
    import concourse.tile as tile
    from concourse import bacc

    f32 = mybir.dt.float32
    f16 = mybir.dt.float16
    Exp = mybir.ActivationFunctionType.Exp
    Copy = mybir.ActivationFunctionType.Copy

    NT = n // 128            # n tiles of 128 (8)
    KP = D // 128            # d_model k-tiles (4)
    HP = HEADS // 2          # head pairs (4)
    T2 = NT // 2             # double-nt chunks (4)

    nc = bacc.Bacc(None, target_bir_lowering=False)

    xT_d = nc.dram_tensor("xT", [bpc, D, n], f16, kind="ExternalInput")
    wq_d = nc.dram_tensor("WqT", [D, D], f16, kind="ExternalInput")   # pre-scaled
    wk_d = nc.dram_tensor("WkT", [D, D], f16, kind="ExternalInput")
    wv_d = nc.dram_tensor("WvT", [D, D], f16, kind="ExternalInput")
    wo_d = nc.dram_tensor("WoT", [D, D], f16, kind="ExternalInput")
    eb_d = nc.dram_tensor("expBT", [HEADS, n, n], f16, kind="ExternalInput")
    out_d = nc.dram_tensor("out", [bpc, n, D], f16, kind="ExternalOutput")

    with tile.TileContext(nc) as tc:
        pers = tc.alloc_tile_pool(name="pers", bufs=1)
        # ---- persistent tiles
        QT = [[pers.tile([128, n], f16, tag=f"qt{bi}_{ip}", name=f"qt{bi}_{ip}")
               for ip in range(KP)] for bi in range(bpc)]
        KT = [[pers.tile([128, n], f16, tag=f"kt{bi}_{ip}", name=f"kt{bi}_{ip}")
               for ip in range(KP)] for bi in range(bpc)]
        VA = [[pers.tile([128, HEADS * (DH + 1)], f16, tag=f"va{bi}_{nt}",
                         name=f"va{bi}_{nt}") for nt in range(NT)]
              for bi in range(bpc)]
        AO = [[pers.tile([128, n], f16, tag=f"ao{bi}_{kp}", name=f"ao{bi}_{kp}")
               for kp in range(KP)] for bi in range(bpc)]
        # each weight matrix lives in one [128, KP, 512] tile (one DMA each)
        # (or per-kp tiles when split_tiles, so first-chunk matmuls can start
        # as soon as their kp slice of W and x has landed)
        w4 = {}
        for wname, wd in (("q", wq_d), ("k", wk_d), ("v", wv_d), ("o", wo_d)):
            if split_tiles:
                w4[wname] = [pers.tile([128, D], f16, tag=f"w{wname}{kp}",
                                       name=f"w{wname}{kp}") for kp in range(KP)]
            else:
                w4[wname] = pers.tile([128, KP, D], f16, tag=f"w{wname}",
                                      name=f"w{wname}")
        xtp = tc.alloc_tile_pool(name="xt", bufs=1)
        if split_tiles:
            xt = [[xtp.tile([128, n], f16, tag=f"x{bi}_{kp}", name=f"x{bi}_{kp}")
                   for kp in range(KP)] for bi in range(bpc)]
        else:
            xt = [xtp.tile([128, KP, n], f16, tag=f"x{bi}", name=f"x{bi}")
                  for bi in range(bpc)]

        def w_ap(wname, kp, sl=slice(None)):
            if split_tiles:
                return w4[wname][kp][:, sl]
            return w4[wname][:, kp, sl]

        def x_ap(bi, kp, sl=slice(None)):
            if split_tiles:
                return xt[bi][kp][:, sl]
            return xt[bi][:, kp, sl]

        ebp = tc.alloc_tile_pool(name="eb", bufs=eb_bufs)
        etp = tc.alloc_tile_pool(name="et", bufs=et_bufs)
        lrp = tc.alloc_tile_pool(name="lr", bufs=lb_bufs)
        fop = tc.alloc_tile_pool(name="fop", bufs=1)
        ps = tc.alloc_tile_pool(name="ps", bufs=1, space="PSUM")

        # ---- prologue DMAs (one chained DMA per tensor)
        def dma_w(wname, wd, kp=None):
            if split_tiles:
                for k in ([kp] if kp is not None else range(KP)):
                    nc.sync.dma_start(out=w4[wname][k][:],
                                      in_=wd[k * 128:(k + 1) * 128, :])
                return
            dst = w4[wname][:]
            src = wd.rearrange("(kp p) c -> p kp c", p=128)
            nc.sync.dma_start(out=dst, in_=src)

        def dma_x(bi, split=False, kp=None):
            if split_tiles:
                for k in ([kp] if kp is not None else range(KP)):
                    nc.sync.dma_start(out=xt[bi][k][:],
                                      in_=xT_d[bi, k * 128:(k + 1) * 128, :])
                return
            if split:
                for kp_ in range(KP):
                    nc.sync.dma_start(
                        out=xt[bi][:, kp_, :],
                        in_=xT_d[bi, kp_ * 128:(kp_ + 1) * 128, :])
            else:
                nc.sync.dma_start(
                    out=xt[bi][:], in_=xT_d[bi].rearrange("(kp p) j -> p kp j", p=128))

        pool_jts = set(range(3, 3 + pool_mul_mod))
        av_jt_order = [j for j in range(NT) if j not in pool_jts] + sorted(pool_jts)
        eb_tiles = {}
        eb_pairs = {}

        def emit_eb_dma(h, jp):
            """Load jt pair (2*jp, 2*jp+1) of head h as one [128, 2, n] tile."""
            t = ebp.tile([128, 2, n], f16, tag="eb", name="eb")
            nc.sync.dma_start(
                out=t[:],
                in_=eb_d[h, 2 * jp * 128:(2 * jp + 2) * 128, :].rearrange(
                    "(two p) i -> p two i", two=2))
            eb_tiles[(h, 2 * jp)] = t[:, 0, :]
            eb_tiles[(h, 2 * jp + 1)] = t[:, 1, :]
            eb_pairs[(h, jp)] = t[:]

        if split_tiles:
            for kp in range(KP):
                dma_w("q", wq_d, kp=kp)
                dma_x(0, kp=kp)
            dma_w("k", wk_d)
            dma_x(1)
            dma_w("v", wv_d)
            dma_w("o", wo_d)
        else:
            dma_w("q", wq_d)
            dma_x(0, split=bool(x0_split))
            dma_w("k", wk_d)
            dma_x(1)
            dma_w("v", wv_d)
            dma_w("o", wo_d)
        for jp in range(NT // 2):
            emit_eb_dma(0, jp)

        # ---- PE warm-up: keep the tensor engine continuously busy through
        # the prologue DMAs so its p-state clock is fully ramped (213ns/row
        # instead of 427+) when the first real projection matmuls arrive.
        if warmup_mms:
            scr = pers.tile([128, 512], f16, tag="scr", name="scr")
            if scr_dve:
                nc.vector.memset(scr[:], 0.0)
            else:
                nc.gpsimd.memset(scr[:], 0.0)
            wps = ps.tile([128, 512], f32, tag="mm", bufs=sim_bufs, name="wps")
            for _ in range(warmup_mms):
                nc.tensor.matmul(wps[:], scr[:, 0:128], scr[:],
                                 start=True, stop=True)

        # ---- emitters -----------------------------------------------------
        inj_ring = [0]
        pend_copies = []

        def flush_copies():
            while pend_copies:
                pend_copies.pop(0)()

        def inj_tag():
            return "ot"

        def emit_qk_chunk(wname, DST, bi, ip, ring=None, halves=(0, 1), first_split=False):
            W_s = w4[wname]
            wide = len(halves) == 2 and not split_inj
            pt = ps.tile([128, 1024 if wide else 512], f32,
                         tag=ring or inj_tag(), bufs=ot_bufs, name="pt")
            for i, nh in enumerate(halves):
                base = i * 512 if wide else 0
                for kp in range(KP):
                    nc.tensor.matmul(
                        pt[:, base:base + 512],
                        w_ap(wname, kp, slice(ip * 128, (ip + 1) * 128)),
                        x_ap(bi, kp, slice(nh * 512, (nh + 1) * 512)),
                        start=(kp == 0), stop=(kp == KP - 1),
                    )
                if not wide:
                    nc.vector.tensor_copy(
                        out=DST[bi][ip][:, nh * 512:(nh + 1) * 512], in_=pt[:])
            if wide:
                if first_split:
                    # sim jt0 needs only K cols 0:128 / Q cols 0:512 - copy
                    # those first so the first sim fires sooner.
                    c0 = 128 if wname == "k" else 512
                    nc.vector.tensor_copy(out=DST[bi][ip][:, 0:c0], in_=pt[:, 0:c0])
                    nc.vector.tensor_copy(out=DST[bi][ip][:, c0:], in_=pt[:, c0:])
                elif defer_qk_copy and ip > 0:
                    pend_copies.append(
                        lambda d=DST[bi][ip], s=pt: nc.vector.tensor_copy(out=d[:], in_=s[:]))
                else:
                    nc.vector.tensor_copy(out=DST[bi][ip][:], in_=pt[:])

        def emit_v_chunk(bi, t2, blocks=(0, 1)):
            wide = len(blocks) == 2 and not split_inj
            pt = ps.tile([128, 1024 if wide else 512], f32, tag=inj_tag(),
                         bufs=ot_bufs, name="pt")
            for i, b in enumerate(blocks):
                nt = 2 * t2 + b
                base = i * 512 if wide else 0
                for kp in range(KP):
                    nc.tensor.matmul(
                        pt[:, base:base + 512],
                        x_ap(bi, kp, slice(nt * 128, (nt + 1) * 128)),
                        w_ap("v", kp),
                        start=(kp == 0), stop=(kp == KP - 1),
                    )
                va = VA[bi][nt]
                nc.gpsimd.memset(va[:], 1.0)
                dst3 = va[:].rearrange("p (h c) -> p h c", c=DH + 1)[:, :, 0:DH]
                src3 = pt[:, base:base + 512].rearrange("p (h c) -> p h c", c=DH)
                nc.vector.tensor_copy(out=dst3, in_=src3)

        def emit_f_drain(pt, bi, nt, b):
            fo = fop.tile([128, 512], f16, tag="fo", bufs=fo_bufs, name="fo")
            if f_copy_dve:
                nc.vector.tensor_copy(out=fo[:], in_=pt[:, b * 512:(b + 1) * 512])
            else:
                nc.scalar.activation(fo[:], pt[:, b * 512:(b + 1) * 512], Copy)
            nc.sync.dma_start(out=out_d[bi, nt * 128:(nt + 1) * 128, :], in_=fo[:])

        def emit_f_chunk(bi, t2, ring=None, fine=False):
            pt = ps.tile([128, 1024], f32, tag=ring or inj_tag(), bufs=ot_bufs,
                         name="pt")
            for b in range(2):
                nt = 2 * t2 + b
                for kp in range(KP):
                    nc.tensor.matmul(
                        pt[:, b * 512:(b + 1) * 512],
                        AO[bi][kp][:, nt * 128:(nt + 1) * 128],
                        w_ap("o", kp),
                        start=(kp == 0), stop=(kp == KP - 1),
                    )
                if fine:
                    emit_f_drain(pt, bi, nt, b)
            if not fine:
                for b in range(2):
                    emit_f_drain(pt, bi, 2 * t2 + b, b)

        def emit_f_nt(bi, nt, ring=None):
            pt = ps.tile([128, 512], f32, tag=ring or inj_tag(), bufs=ot_bufs, name="ptf")
            for kp in range(KP):
                nc.tensor.matmul(
                    pt[:, 0:512],
                    AO[bi][kp][:, nt * 128:(nt + 1) * 128],
                    w_ap("o", kp),
                    start=(kp == 0), stop=(kp == KP - 1),
                )
            emit_f_drain(pt, bi, nt, 0)

        pend_avs = []
        pend_norms = []

        def pend_any():
            return bool(pend_avs)

        def queue_norm(h, bi, ot):
            if norm_delay and (h // 2) != HP - 1:
                pend_norms.append((h, bi, ot))
            else:
                flush_norms()
                emit_norm(h, bi, ot)

        def flush_norms():
            while pend_norms:
                emit_norm(*pend_norms.pop(0))

        def start_av(h, bi, et_row, inline=False):
            ot = ps.tile([DH + 1, 1024], f32, tag="ot", bufs=ot_bufs, name="ot")
            if inline:
                mms = [(ihh, jt) for jt in av_jt_order for ihh in range(2)]
            else:
                mms = [(ihh, jt) for ihh in range(2) for jt in av_jt_order]
            pend_avs.append({"h": h, "bi": bi, "et": et_row, "ot": ot,
                             "mms": mms, "lag": inline_av_lag if inline else 0})

        def step_av(nmm):
            while nmm > 0 and pend_avs:
                st = pend_avs[0]
                h, bi, et_row, ot = st["h"], st["bi"], st["et"], st["ot"]
                if not st["mms"]:
                    pend_avs.pop(0)
                    queue_norm(h, bi, ot)
                    continue
                ihh, jt = st["mms"][0]
                lag = st["lag"] if st is pend_avs[-1] else 0
                if jt >= len(et_row) - lag:
                    return
                st["mms"].pop(0)
                nc.tensor.matmul(
                    ot[:, ihh * 512:(ihh + 1) * 512],
                    VA[bi][jt][:, h * (DH + 1):(h + 1) * (DH + 1)],
                    et_row[jt][:, ihh * 512:(ihh + 1) * 512],
                    start=(jt == av_jt_order[0]), stop=(jt == av_jt_order[-1]),
                )
                nmm -= 1
                if not st["mms"]:
                    pend_avs.pop(0)
                    queue_norm(h, bi, ot)

        norm_idx = [0]

        def emit_norm(h, bi, ot, half=None):
            if norm_fast:
                # Short chain: cross-partition-base DVE copy moves the psum
                # l-row (partition 64) straight to a partition-0 SBUF row (no
                # DMA hop), reciprocal there, broadcast, and the apply writes
                # odd-head AO rows at partition base 64 directly (no tmpo DMA).
                if isinstance(half, tuple):
                    idx, cnt = half
                    w = n // cnt
                    sl = slice(idx * w, (idx + 1) * w)
                    half = 0 if idx == 0 else 1  # tile alloc on first piece only
                elif half is not None:
                    sl = slice(half * 512, (half + 1) * 512)
                else:
                    sl = slice(None)
                lbf = lrp.tile([DH, n], f32, tag="lb", name="lbf") if half != 1 \
                    else emit_norm.lb_cur
                emit_norm.lb_cur = lbf
                lrow = lrp.tile([1, n], f32, tag="lrow", bufs=2, name="lrow") if half != 1 \
                    else emit_norm.lrow_cur
                emit_norm.lrow_cur = lrow
                nc.vector.tensor_copy(out=lrow[0:1, sl], in_=ot[DH:DH + 1, sl])
                if recip_scatter and half is None:
                    nsc = n // 128
                    lr0s = lrp.tile([128, nsc], f32, tag="lr0s", bufs=2, name="lr0s")
                    nc.sync.dma_start(out=lr0s[:], in_=lrow[0:1, :])
                    nc.vector.reciprocal_approx_fast(out=lr0s[:], in_=lr0s[:])
                    nc.sync.dma_start(out=lrow[0:1, :], in_=lr0s[:])
                else:
                    nc.vector.reciprocal_approx_fast(out=lrow[0:1, sl], in_=lrow[0:1, sl])
                nc.gpsimd.partition_broadcast(lbf[0:DH, sl], lrow[0:1, sl], channels=DH)
                po = (h % 2) * DH
                nc.vector.tensor_mul(
                    out=AO[bi][h // 2][po:po + DH, sl], in0=ot[0:DH, sl],
                    in1=lbf[0:DH, sl])
                return
            # HW-validated chain: copy the psum l-row (partition 64) to SBUF,
            # DMA it to partition 0, reciprocal there, broadcast from
            # partition 0. Custom DVE ops and partition_broadcast only work
            # from partition 0 on real hardware; ACT can read psum anywhere.
            sl = slice(None) if half is None else slice(half * 512, (half + 1) * 512)
            w = n if half is None else 512
            lb = lrp.tile([DH + 1, n], f32, tag="lb", name="lb") if half != 1 \
                else emit_norm.lb_cur
            emit_norm.lb_cur = lb
            if norm_idx[0] % 2 < row_copy_act:
                nc.scalar.activation(lb[DH:DH + 1, sl], ot[DH:DH + 1, sl], Copy)
            else:
                nc.vector.tensor_copy(out=lb[DH:DH + 1, sl], in_=ot[DH:DH + 1, sl])
            norm_idx[0] += 1
            if recip_scatter and half is None:
                # scatter the l row across 128 partitions, reciprocal there
                # (128x less DVE row time), gather back to a partition-0 row.
                nsc = n // 128
                lr0s = lrp.tile([128, nsc], f32, tag="lr0s", bufs=1, name="lr0s")
                nc.sync.dma_start(out=lr0s[:], in_=lb[DH:DH + 1, :])
                lrrs = lrp.tile([128, nsc], f32, tag="lrrs", bufs=1, name="lrrs")
                nc.vector.reciprocal_approx_fast(out=lrrs[:], in_=lr0s[:])
                lrr = lrp.tile([1, n], f32, tag="lrr", bufs=1, name="lrr")
                nc.sync.dma_start(out=lrr[0:1, :], in_=lrrs[:])
            else:
                lr0 = lrp.tile([1, n], f32, tag="lr0", bufs=1, name="lr0") if half != 1 \
                    else emit_norm.lr0_cur
                emit_norm.lr0_cur = lr0
                nc.sync.dma_start(out=lr0[0:1, sl], in_=lb[DH:DH + 1, sl])
                lrr = lrp.tile([1, n], f32, tag="lrr", bufs=1, name="lrr") if half != 1 \
                    else emit_norm.lrr_cur
                emit_norm.lrr_cur = lrr
                nc.vector.reciprocal_approx_fast(out=lrr[0:1, sl], in_=lr0[0:1, sl])
            nc.gpsimd.partition_broadcast(lb[0:DH, sl], lrr[0:1, sl], channels=DH)
            if h % 2 == 0:
                nc.vector.tensor_mul(
                    out=AO[bi][h // 2][0:DH, sl], in0=ot[0:DH, sl], in1=lb[0:DH, sl])
            else:
                tmpo = lrp.tile([DH, n], f16, tag="tmpo", bufs=tmpo_bufs, name="tmpo") \
                    if half != 1 else emit_norm.tmpo_cur
                emit_norm.tmpo_cur = tmpo
                nc.vector.tensor_mul(out=tmpo[0:DH, sl], in0=ot[0:DH, sl], in1=lb[0:DH, sl])
                nc.sync.dma_start(out=AO[bi][h // 2][DH:128, sl], in_=tmpo[0:DH, sl])

        # ---- prologue compute: QK projections for ip=0 (heads 0,1).
        # The sim ring is empty this early, so borrow it: four chunks then
        # rotate through four psum slots instead of two.
        for bi in range(bpc):
            emit_qk_chunk("q", QT, bi, 0, ring="mm" if proq_mm else None,
                          first_split=(bi == 0 and bool(head_split)))
            emit_qk_chunk("k", KT, bi, 0, ring="mm" if proq_mm else None,
                          first_split=(bi == 0 and bool(head_split)))

        # ---- deadline-scheduled inject: chunk -> emission slot
        sched_ip1 = list(sched_ip1)
        sched = {}
        slots_v0 = [0, 2, 4, 6]
        slots_v1 = [8, 10, 12, 14]
        slots_ip = {1: sched_ip1, 2: list(sched_ip2), 3: list(sched_ip3)}
        if split_inj:
            for t2 in range(T2):
                sched[slots_v0[t2]] = [("vh", 0, t2, 0), ("vh", 0, t2, 1)]
                sched[slots_v1[t2]] = [("vh", 1, t2, 0), ("vh", 1, t2, 1)]
            for ip in range(1, KP):
                for i, (kind, bi) in enumerate((("q", 0), ("k", 0), ("q", 1), ("k", 1))):
                    sched[slots_ip[ip][i]] = [(kind + "h", bi, ip, 0),
                                              (kind + "h", bi, ip, 1)]
        else:
            for t2 in range(T2):
                sched[slots_v0[t2]] = [("v", 0, t2)]
                sched[slots_v1[t2]] = [("v", 1, t2)]
            for ip in range(1, KP):
                for i, (kind, bi) in enumerate((("q", 0), ("k", 0), ("q", 1), ("k", 1))):
                    sched[slots_ip[ip][i]] = [(kind, bi, ip)]
        if f0_fine:
            for i, s in enumerate(sched_f0):
                sched.setdefault(s, []).append(("ff", 0, i))
        else:
            for i, t2 in enumerate(range(T2)):
                sched.setdefault(sched_f0[i], []).append(("f", 0, t2))

        def emit_chunk(c):
            kind = c[0]
            if kind == "v":
                emit_v_chunk(c[1], c[2])
            elif kind == "vh":
                emit_v_chunk(c[1], c[2], blocks=(c[3],))
            elif kind == "q":
                emit_qk_chunk("q", QT, c[1], c[2])
            elif kind == "k":
                emit_qk_chunk("k", KT, c[1], c[2])
            elif kind == "qh":
                emit_qk_chunk("q", QT, c[1], c[2], halves=(c[3],))
            elif kind == "kh":
                emit_qk_chunk("k", KT, c[1], c[2], halves=(c[3],))
            elif kind == "f":
                flush_norms()
                emit_f_chunk(c[1], c[2])
            elif kind == "ff":
                flush_norms()
                emit_f_nt(c[1], c[2])

        # ---- main attention loop; last head pair runs h-odd first so the
        # final stream is an even head (its norm-mul writes AO directly,
        # shortening the tail by one SBUF->SBUF DMA hop).
        stream_order = {}
        for hp in range(HP):
            h0, h1 = 2 * hp, 2 * hp + 1
            if hp == HP - 1:
                if hp3_order:
                    stream_order[hp] = [(h1, 0), (h0, 0), (h1, 1), (h0, 1)]
                else:
                    stream_order[hp] = [(h1, 0), (h1, 1), (h0, 0), (h0, 1)]
            else:
                stream_order[hp] = [(h0, 0), (h0, 1), (h1, 0), (h1, 1)]

        slot = 0
        for hp in range(HP):
            streams = stream_order[hp]
            first_h = streams[0][0]
            other_h = streams[2][0]
            for sidx, (h, bi) in enumerate(streams):
                is_last = (hp == HP - 1 and sidx == 3)
                ot_last = None
                if is_last:
                    ot_last = ps.tile([DH + 1, 1024], f32, tag="ot",
                                      bufs=ot_bufs, name="ot_last")
                et_row = []
                stream_state = {}
                if inline_av_lag and not is_last:
                    start_av(h, bi, et_row, inline=True)
                sp_hoist = [None]
                for jt in range(NT):
                    # eb prefetch (jt pairs): sidx1 fetches this pair's other
                    # head, sidx2 fetches the next head-pair's first head.
                    if jt % 2 == 0:
                        if hp3_order and hp == HP - 1:
                            if sidx == 0:
                                emit_eb_dma(streams[1][0], jt // 2)
                        elif sidx == 1:
                            emit_eb_dma(other_h, jt // 2)
                        if sidx == 2 and hp + 1 < HP:
                            if hp3_order and hp + 1 == HP - 1:
                                pass
                            emit_eb_dma(stream_order[hp + 1][0][0], jt // 2)
                    po = (h % 2) * 64

                    def emit_sim(j):
                        tag = "ot" if (sim7_ot and j >= NT - sim7_ot) else "mm"
                        spj = ps.tile([128, 1024], f32, tag=tag,
                                      bufs=(ot_bufs if tag == "ot" else sim_bufs),
                                      name="spj")
                        for ihh in range(2):
                            nc.tensor.matmul(
                                spj[:, ihh * 512:(ihh + 1) * 512],
                                KT[bi][hp][po:po + 64, j * 128:(j + 1) * 128],
                                QT[bi][hp][po:po + 64, ihh * 512:(ihh + 1) * 512],
                                start=True, stop=True,
                            )
                        return spj

                    if av_first and jt <= 1 and not inline_av_lag:
                        # ready av matmuls of the previous stream go ahead of
                        # the boundary sims (which wait on ACT freeing the
                        # sim psum ring) to avoid in-order head blocking.
                        step_av(av_first)
                    if jt == 1 and sp_hoist[0] is not None:
                        sp = sp_hoist[0]
                    else:
                        sp = emit_sim(jt)
                    if hoist_sim and jt == 0 and pend_any():
                        sp_hoist[0] = emit_sim(1)
                    if pair_mul:
                        if jt % 2 == 0:
                            eqp = etp.tile([128, 2, n], f16, tag="eq",
                                           bufs=max(2, eq_bufs // 2), name="eqp")
                            stream_state["eqp"] = eqp
                            nc.scalar.activation(eqp[:, 0, :], sp[:], Exp)
                        else:
                            eqp = stream_state["eqp"]
                            nc.scalar.activation(eqp[:, 1, :], sp[:], Exp)
                            etp2 = etp.tile([128, 2, n], f16, tag="et",
                                            bufs=max(2, et_bufs // 2), name="etp2")
                            nc.vector.tensor_mul(out=etp2[:], in0=eqp[:],
                                                 in1=eb_pairs[(h, jt // 2)])
                            et_row.append(etp2[:, 0, :])
                            et_row.append(etp2[:, 1, :])
                    else:
                        eq = etp.tile([128, n], f16, tag="eq", bufs=eq_bufs, name="eq")
                        nc.scalar.activation(eq[:], sp[:], Exp)
                        et = etp.tile([128, n], f16, tag="et", name="et")
                        use_pool = (not is_last) and jt in pool_jts
                        mul_eng = nc.gpsimd if use_pool else nc.vector
                        mul_eng.tensor_mul(out=et[:], in0=eq[:], in1=eb_tiles[(h, jt)])
                        et_row.append(et)
                    if is_last:
                        if pair_mul:
                            if jt % 2 == 1:
                                for j2 in (jt - 1, jt):
                                    for ihh in range(2):
                                        nc.tensor.matmul(
                                            ot_last[:, ihh * 512:(ihh + 1) * 512],
                                            VA[bi][j2][:, h * (DH + 1):(h + 1) * (DH + 1)],
                                            et_row[j2][:, ihh * 512:(ihh + 1) * 512],
                                            start=(j2 == 0), stop=(j2 == NT - 1),
                                        )
                        else:
                            for ihh in range(2):
                                nc.tensor.matmul(
                                    ot_last[:, ihh * 512:(ihh + 1) * 512],
                                    VA[bi][jt][:, h * (DH + 1):(h + 1) * (DH + 1)],
                                    et[:, ihh * 512:(ihh + 1) * 512],
                                    start=(jt == 0), stop=(jt == NT - 1),
                                )
                    flush_copies()
                    if jt == av_flush_slot and not inline_av_lag:
                        step_av(99)
                    if slot in sched:
                        for c in sched.pop(slot):
                            emit_chunk(c)
                    if inline_av_lag:
                        step_av(6)
                    if norm_delay and jt == norm_delay:
                        flush_norms()
                    slot += 1
                if not inline_av_lag:
                    step_av(99)  # flush any remaining AV of the previous stream
                if is_last:
                    if inline_av_lag:
                        step_av(99)  # drain all pending AV before the tail norm
                    flush_norms()
                    if last_norm_split >= 2:
                        npc = 2 * last_norm_split
                        for qi in range(npc):
                            emit_norm(h, bi, ot_last, half=(qi, npc))
                    elif last_norm_split:
                        emit_norm(h, bi, ot_last, half=0)
                        emit_norm(h, bi, ot_last, half=1)
                    else:
                        emit_norm(h, bi, ot_last)
                elif not inline_av_lag:
                    start_av(h, bi, et_row)
        step_av(99)
        flush_norms()
        # drain: anything left, then F for batch 1
        for s in sorted(sched):
            for c in sched[s]:
                emit_chunk(c)
        if drain_pre and drain_nt:
            # Pre-accumulate kp0..2 of the first drain chunks into free psum
            # slots while PE would otherwise idle waiting the last norm-apply;
            # only the ao-kp3-dependent matmul + drain remain serial.
            pts = {}

            def pre_nt(nt, ring):
                pt = ps.tile([128, 512], f32, tag=ring, bufs=ot_bufs, name="ptp")
                pts[nt] = pt
                for kp in range(KP - 1):
                    nc.tensor.matmul(
                        pt[:, 0:512],
                        AO[1][kp][:, nt * 128:(nt + 1) * 128],
                        w_ap("o", kp),
                        start=(kp == 0), stop=False)

            def fin_nt(nt):
                pt = pts.pop(nt)
                nc.tensor.matmul(
                    pt[:, 0:512],
                    AO[1][KP - 1][:, nt * 128:(nt + 1) * 128],
                    w_ap("o", KP - 1),
                    start=False, stop=True)
                emit_f_drain(pt, 1, nt, 0)

            for i, nt in enumerate(range(4)):
                pre_nt(nt, "mm" if i < 2 else "ot")
            for i, nt in enumerate(range(4)):
                fin_nt(nt)
                pre_nt(nt + 4, "mm" if i < 2 else "ot")
            for nt in range(4, NT):
                fin_nt(nt)
        elif drain_nt:
            for nt in range(NT):
                emit_f_nt(1, nt, ring=("mm" if (f_drain_mm and nt % 2) else None))
        else:
            for t2 in range(T2):
                emit_f_chunk(1, t2, ring=("mm" if (f_drain_mm and t2 % 2) else None),
                             fine=bool(drain_fine) and t2 == T2 - 1)

        for p in (ps, fop, lrp, etp, ebp, xtp, pers):
            p.release()

    nc.compile()
    return nc


def prep_inputs(x, Wq, Wk, Wv, rel_bias, Wo, bo, n=N, bpc=BPC, ncores=NCORES):
    """Host-side sharding/layout prep. Returns in_maps (one dict per core)."""
    f16 = np.float16
    x = np.asarray(x, dtype=np.float32)
    xT = np.ascontiguousarray(x.transpose(0, 2, 1)).astype(f16)   # [B, D, n]
    WqT = np.ascontiguousarray(Wq.T * np.float32(SCALE)).astype(f16)
    WkT = np.ascontiguousarray(Wk.T).astype(f16)
    WvT = np.ascontiguousarray(Wv.T).astype(f16)
    WoT = np.ascontiguousarray(Wo.T).astype(f16)
    expBT = np.ascontiguousarray(
        np.exp(np.asarray(rel_bias, dtype=np.float32).transpose(0, 2, 1))
    ).astype(f16)                                                  # [H, n(j), n(i)]
    in_maps = []
    for c in range(ncores):
        in_maps.append({
            "xT": np.ascontiguousarray(xT[c * bpc:(c + 1) * bpc]),
            "WqT": WqT, "WkT": WkT, "WvT": WvT, "WoT": WoT,
            "expBT": expBT,
        })
    return in_maps


BEST_KW = dict(norm_fast=1, f_drain_mm=1, norm_delay=6, hp3_order=1,
               f0_fine=1, sched_f0=(113, 114, 115, 117, 119, 121, 123, 125),
               eb_bufs=8, drain_nt=1, eq_bufs=10, et_bufs=16, last_norm_split=1,
               x0_split=1, sched_ip1=(18, 22, 26, 30), av_first=1)
_CACHE = {}


def kernel(x, Wq, Wk, Wv, rel_bias, Wo, bo):
    from concourse.bass_utils import run_bass_kernel_spmd

    if "nc" not in _CACHE:
        _CACHE["nc"] = build_nc(**BEST_KW)
    nc = _CACHE["nc"]
    in_maps = prep_inputs(x, Wq, Wk, Wv, rel_bias, Wo, bo)
    res = run_bass_kernel_spmd(nc, in_maps, core_ids=list(range(NCORES)))
    out = np.concatenate([res.results[c]["out"] for c in range(NCORES)], axis=0)
    out = out + np.asarray(bo, dtype=np.float32)[None, None, :]
    return np.ascontiguousarray(out, dtype=np.float32)

